# revision 21
# baseline (speedup 1.0000x reference)
"""Trainium2 Bass kernel for nn_DQN_34136400069239 (DeepSets-style pooling).

Math (reference):
    h1  = relu(x @ pw1 + pb1)          [N, H]
    h2  = relu(h1 @ pw2 + pb2)         [N, H]
    phi = h2 @ pw3 + pb3               [N, F]
    fp  = sum(phi, axis=0)             [F]
    ... tiny rho MLP + concat(x_static) + tiny 3-layer MLP -> [OUT]

The third phi layer is linear, so fp = (sum_n h2[n]) @ pw3 + N * pb3 and the
device only computes S = sum_n relu(h2[n]) in R^H.  Data-parallel over rows:
8 cores x 50000 rows, host sums the 8 partial S vectors and runs the tail.

Device design (mode "v3*"), per 1000-row pair of 500-row blocks:
  - PSUM is laid out pair-level and half-major: ps1_h0/ps1_h1 and
    ps2_h0/ps2_h1 are [128, 2(block), 512] tiles.  Each vector-engine op
    then covers ONE h-half of TWO blocks, so its per-partition bias vector
    is uniform and accum_out keeps per-channel sums:
      DVE: h1 = max(ps1_h + b1_h, 0)   tensor_scalar(add, max), FD=1000
      ACT: relu(ps2_h + b2_h) + row-sum accum_out, FD=1000
    Those two engines are the roofline (~1167 / ~977 ns per 500-row block);
    biases ride along for free as exact-f32 per-partition operands.
  - mode "v3"    : layer 2 = 4 f16 matmuls (K=128) per block, PE ~3000 c/blk
  - mode "v3fp8" : layer 2 = 2 fp8e4m3 DoubleRow matmuls (K_eff=256) per
                   block, PE ~2100 c/blk; W2 quantized with error diffusion
                   down the contraction axis (plain fp8 rounding fails the
                   2e-2 gate at 2.3e-2; diffusion passes at ~3.5e-3).
"""

import os

import numpy as np

# Problem constants (hardcoded; kernel.py must be self-contained).
N = 400000
IN, H, F, S_STATIC, OUT = 64, 256, 128, 16, 5
N_CORES = 8
R = N // N_CORES  # rows per core = 50000
BLK = 500  # matmul moving free dim
NBLK = R // BLK  # 100
NPAIR = NBLK // 2  # 50

MODE = os.environ.get("DQN_MODE", "v5")

_prog_cache: dict = {}


def _build(mode: str, iters: int = 1):
    if mode == "v4":
        return _build_v4(iters)
    if mode == "v5":
        return _build_v5(iters)
    if mode == "v6":
        return _build_v6(iters)
    import concourse.mybir as mybir
    import concourse.tile as tile
    from concourse import bacc
    from contextlib import ExitStack

    dt = mybir.dt
    f32 = dt.float32
    f16 = dt.float16
    fp8 = mode == "v3fp8"
    h1_dt = dt.float8e4 if fp8 else f16

    nc = bacc.Bacc(
        "TRN2",
        target_bir_lowering=False,
        debug=False,
        enable_asserts=False,
        num_devices=1,
    )

    d_xt = nc.dram_tensor("d_xt", [IN, R], f16, kind="ExternalInput").ap()
    d_w1 = nc.dram_tensor("d_w1", [IN, H], f16, kind="ExternalInput").ap()
    if fp8:
        # packed [k, pair, m]: W2p[k, i, m] = W2q[128*i + k, m]
        d_w2 = nc.dram_tensor("d_w2", [128, 2, H], dt.float8e4, kind="ExternalInput").ap()
    else:
        d_w2 = nc.dram_tensor("d_w2", [H, H], f16, kind="ExternalInput").ap()
    # f32 per-partition biases: cols = [b1_h0, b1_h1, b2_h0, b2_h1]
    d_b = nc.dram_tensor("d_b", [128, 4], f32, kind="ExternalInput").ap()
    d_s = nc.dram_tensor("d_s", [128, 2], f32, kind="ExternalOutput").ap()

    Relu = mybir.ActivationFunctionType.Relu
    Alu = mybir.AluOpType
    X = mybir.AxisListType.X

    with tile.TileContext(nc) as tc, ExitStack() as ctx:
        cpool = ctx.enter_context(tc.tile_pool(name="cpool", bufs=1))
        xpool = ctx.enter_context(tc.tile_pool(name="xpool", bufs=3))
        hpool = ctx.enter_context(tc.tile_pool(name="hpool", bufs=2))
        spool = ctx.enter_context(tc.tile_pool(name="spool", bufs=1))
        ps1p = ctx.enter_context(tc.tile_pool(name="ps1p", bufs=1, space="PSUM"))
        ps2p = ctx.enter_context(tc.tile_pool(name="ps2p", bufs=1, space="PSUM"))

        # Constants resident in SBUF.
        w1_sb = cpool.tile([IN, H], f16, name="w1_sb")
        nc.sync.dma_start(w1_sb[:], d_w1)
        if fp8:
            w2p_sb = cpool.tile([128, 2, H], dt.float8e4, name="w2p_sb")
            nc.sync.dma_start(w2p_sb[:], d_w2)
        else:
            w2_sb = []
            for k in range(2):
                t = cpool.tile([128, H], f16, name=f"w2_sb{k}")
                nc.sync.dma_start(t[:], d_w2[k * 128 : (k + 1) * 128, :])
                w2_sb.append(t)
        bv = cpool.tile([128, 4], f32, name="bv")
        nc.sync.dma_start(bv[:], d_b)

        # Per-pair accumulated row-sums of relu(h2), one column per pair.
        acc = [cpool.tile([128, NPAIR], f32, name=f"acc{m}") for m in range(2)]

        for pair in [p for _ in range(iters) for p in range(NPAIR)]:
            xt = xpool.tile([IN, 2 * BLK], f16, name="xt", tag="xt")
            nc.sync.dma_start(xt[:], d_xt[:, pair * 2 * BLK : (pair + 1) * 2 * BLK])

            ps1 = [
                ps1p.tile([128, 2, 512], f32, name=f"ps1_{m}", tag=f"ps1_{m}")
                for m in range(2)
            ]
            ps2 = [
                ps2p.tile([128, 2, 512], f32, name=f"ps2_{m}", tag=f"ps2_{m}")
                for m in range(2)
            ]

            # Layer 1: 4 K=64 matmuls into half-major pair psum.
            for j in range(2):
                xr = xt[:, j * BLK : (j + 1) * BLK]
                for m in range(2):
                    nc.tensor.matmul(
                        ps1[m][:, j, 0:BLK],
                        w1_sb[:, m * 128 : (m + 1) * 128],
                        xr,
                        start=True,
                        stop=True,
                    )

            # h1 = relu(ps1 + b1): one DVE op per half (uniform bias vector).
            h1 = hpool.tile([128, 2, 2, 512], h1_dt, name="h1", tag="h1")
            for m in range(2):
                nc.vector.tensor_scalar(
                    h1[:, m, :, 0:BLK],
                    ps1[m][:, :, 0:BLK],
                    bv[:, m : m + 1],
                    0.0,
                    op0=Alu.add,
                    op1=Alu.max,
                )

            # Layer 2 into pair-level psum.
            for j in range(2):
                if fp8:
                    for m in range(2):
                        nc.tensor.matmul(
                            ps2[m][:, j, 0:BLK],
                            w2p_sb[:, :, m * 128 : (m + 1) * 128],
                            h1[:, :, j, 0:BLK],
                            start=True,
                            stop=True,
                            perf_mode=mybir.MatmulPerfMode.DoubleRow,
                        )
                else:
                    for m in range(2):
                        for k in range(2):
                            nc.tensor.matmul(
                                ps2[m][:, j, 0:BLK],
                                w2_sb[k][:, m * 128 : (m + 1) * 128],
                                h1[:, k, j, 0:BLK],
                                start=(k == 0),
                                stop=(k == 1),
                            )

            # relu(ps2 + b2) with fused row-sum; channels preserved because
            # each op spans one half of both blocks.
            for m in range(2):
                scr = spool.tile([128, 2, 512], f16, name=f"scr{m}", tag=f"scr{m}")
                nc.scalar.activation(
                    scr[:, :, 0:BLK],
                    ps2[m][:, :, 0:BLK],
                    Relu,
                    bias=bv[:, 2 + m : 3 + m],
                    accum_out=acc[m][:, pair : pair + 1],
                )

        s_sb = cpool.tile([128, 2], f32, name="s_sb")
        for m in range(2):
            nc.vector.reduce_sum(s_sb[:, m : m + 1], acc[m][:], axis=X)
        nc.sync.dma_start(d_s, s_sb[:])

    nc.compile()
    return nc


def _build_v4(iters: int = 1):
    """ACT-centric fp8 variant.

    Empirical per-op costs (probe.py, chained, psum-f32 src):
      ACT  = ~383 + 0.25*FD ns   (f16 out; 4x-packed stream)
      DVE  = ~397 + 0.71*FD ns
    so ACT is the cheap drain and op count is what matters.  Per 1000-row
    pair: ONE ACT op does relu(ps1) for all four [half,block] layer-1 banks
    (FD=2000; b1 pre-added by K=1 ones-matmuls on PE strips 2-3, concurrent
    with the K=64 layer-1 matmuls on strips 0-1); layer-2 relu+accum runs
    half0 on ACT, half1 on DVE (bias as per-partition vector operands).
    Layer 2 is 2 fp8 DoubleRow matmuls per block (K_eff=256), weights-outer
    so LDWEIGHTS amortizes over the pair.
    """
    import concourse.mybir as mybir
    import concourse.tile as tile
    from concourse import bacc
    from contextlib import ExitStack

    dt = mybir.dt
    f32 = dt.float32
    f16 = dt.float16
    fp8 = dt.float8e4
    Relu = mybir.ActivationFunctionType.Relu
    Alu = mybir.AluOpType
    X = mybir.AxisListType.X

    nc = bacc.Bacc(
        "TRN2",
        target_bir_lowering=False,
        debug=False,
        enable_asserts=False,
        num_devices=1,
    )

    d_xt = nc.dram_tensor("d_xt", [IN, R], f16, kind="ExternalInput").ap()
    d_w1 = nc.dram_tensor("d_w1", [IN, H], f16, kind="ExternalInput").ap()
    d_w2 = nc.dram_tensor("d_w2", [128, 2, H], fp8, kind="ExternalInput").ap()
    # f16 b1 halves for the ones-matmuls, rows 64/96; f32 b2 via vector ops.
    d_b1 = nc.dram_tensor("d_b1", [128, 128], f16, kind="ExternalInput").ap()
    d_b2 = nc.dram_tensor("d_b2", [128, 2], f32, kind="ExternalInput").ap()
    d_s = nc.dram_tensor("d_s", [128, 2], f32, kind="ExternalOutput").ap()

    with tile.TileContext(nc) as tc, ExitStack() as ctx:
        cpool = ctx.enter_context(tc.tile_pool(name="cpool", bufs=1))
        xpool = ctx.enter_context(tc.tile_pool(name="xpool", bufs=3))
        hpool = ctx.enter_context(tc.tile_pool(name="hpool", bufs=2))
        spool = ctx.enter_context(tc.tile_pool(name="spool", bufs=1))
        ps1p = ctx.enter_context(tc.tile_pool(name="ps1p", bufs=1, space="PSUM"))
        ps2p = ctx.enter_context(tc.tile_pool(name="ps2p", bufs=1, space="PSUM"))

        w1_sb = cpool.tile([IN, H], f16, name="w1_sb")
        nc.sync.dma_start(w1_sb[:], d_w1)
        w2p_sb = cpool.tile([128, 2, H], fp8, name="w2p_sb")
        nc.sync.dma_start(w2p_sb[:], d_w2)
        b1_sb = cpool.tile([128, 128], f16, name="b1_sb")
        nc.sync.dma_start(b1_sb[:], d_b1)
        b2_sb = cpool.tile([128, 2], f32, name="b2_sb")
        nc.sync.dma_start(b2_sb[:], d_b2)
        nb2 = cpool.tile([128, 1], f32, name="nb2")
        nc.vector.tensor_scalar_mul(nb2[:], b2_sb[:, 1:2], -1.0)
        ones_sb = cpool.tile([128, BLK], f16, name="ones_sb")
        nc.vector.memset(ones_sb[:], 1.0)

        acc = [cpool.tile([128, NPAIR], f32, name=f"acc{m}") for m in range(2)]

        for pair in [p for _ in range(iters) for p in range(NPAIR)]:
            xt = xpool.tile([IN, 2 * BLK], f16, name="xt", tag="xt")
            nc.sync.dma_start(xt[:], d_xt[:, pair * 2 * BLK : (pair + 1) * 2 * BLK])

            # ps1: [half, block] banks, 4 banks, one tile per pair.
            ps1 = ps1p.tile([128, 2, 2, 512], f32, name="ps1", tag="ps1")
            for j in range(2):
                xr = xt[:, j * BLK : (j + 1) * BLK]
                for m in range(2):
                    strip = 64 if m == 0 else 96
                    nc.tensor.matmul(
                        ps1[:, m, j, 0:BLK],
                        b1_sb[strip : strip + 1, 0:128],
                        ones_sb[strip : strip + 1, 0:BLK],
                        start=True,
                        stop=False,
                        tile_position=(strip, 0),
                        skip_group_check=True,
                    )
                    nc.tensor.matmul(
                        ps1[:, m, j, 0:BLK],
                        w1_sb[:, m * 128 : (m + 1) * 128],
                        xr,
                        start=False,
                        stop=True,
                        skip_group_check=True,
                    )

            # (a): one ACT op drains all of ps1 -> packed fp8 h1.
            h1 = hpool.tile([128, 2, 2, 512], fp8, name="h1", tag="h1")
            nc.scalar.activation(h1[:, :, :, 0:BLK], ps1[:, :, :, 0:BLK], Relu)

            # Layer 2: DoubleRow, weights-outer so each half's LDWEIGHTS is
            # shared by both blocks of the pair.
            ps2 = [
                ps2p.tile([128, 2, 512], f32, name=f"ps2_{m}", tag=f"ps2_{m}")
                for m in range(2)
            ]
            for m in range(2):
                for j in range(2):
                    nc.tensor.matmul(
                        ps2[m][:, j, 0:BLK],
                        w2p_sb[:, :, m * 128 : (m + 1) * 128],
                        h1[:, :, j, 0:BLK],
                        start=True,
                        stop=True,
                        perf_mode=mybir.MatmulPerfMode.DoubleRow,
                    )

            # (b): relu(ps2 + b2) + per-channel row-sum; half0 on ACT,
            # half1 on DVE so the two drains run in parallel.
            scr0 = spool.tile([128, 2, 512], f16, name="scr0", tag="scr0")
            nc.scalar.activation(
                scr0[:, :, 0:BLK],
                ps2[0][:, :, 0:BLK],
                Relu,
                bias=b2_sb[:, 0:1],
                accum_out=acc[0][:, pair : pair + 1],
            )
            scr1 = spool.tile([128, 2, 512], f16, name="scr1", tag="scr1")
            nc.vector.tensor_scalar(
                scr1[:, :, 0:BLK],
                ps2[1][:, :, 0:BLK],
                nb2[:],
                None,
                op0=Alu.max,
                op1=Alu.add,
                accum_out=acc[1][:, pair : pair + 1],
            )

        s_sb = cpool.tile([128, 2], f32, name="s_sb")
        for m in range(2):
            nc.vector.reduce_sum(s_sb[:, m : m + 1], acc[m][:], axis=X)
        nc.sync.dma_start(d_s, s_sb[:])

    nc.compile()
    return nc


def _build_v5(iters: int = 1):
    """fp8 DoubleRow layer 2 with probe-informed engine split.

    Empirical per-op costs (probe.py, chained, psum-f32 src, FD=1000):
      ACT relu+bias(+accum) ~633 ns ;  DVE 2-op(+accum) ~1267 ns
    Per 1000-row pair (ops all pair-level, half-major so the per-partition
    bias vector is uniform within each op):
      ACT: (a)h0, (a)h1  relu(ps1+b1)->fp8 h1,  (b)h0 relu+accum  ~1.9 us
      DVE: (b)h1 relu+accum                                       ~1.3 us
      PE : 4x K=64 f16 layer-1 mm + 4x DoubleRow K_eff=256 layer-2 mm
           (weights-outer so each half's LDWEIGHTS covers both blocks)
    """
    import concourse.mybir as mybir
    import concourse.tile as tile
    from concourse import bacc
    from contextlib import ExitStack

    dt = mybir.dt
    f32 = dt.float32
    f16 = dt.float16
    fp8 = dt.float8e4
    Relu = mybir.ActivationFunctionType.Relu
    Alu = mybir.AluOpType
    X = mybir.AxisListType.X

    nc = bacc.Bacc(
        "TRN2",
        target_bir_lowering=False,
        debug=False,
        enable_asserts=False,
        num_devices=1,
    )

    d_xt = nc.dram_tensor("d_xt", [IN, R], f16, kind="ExternalInput").ap()
    d_w1 = nc.dram_tensor("d_w1", [IN, H], f16, kind="ExternalInput").ap()
    d_w2 = nc.dram_tensor("d_w2", [128, 2, H], fp8, kind="ExternalInput").ap()
    # f32 per-partition biases: cols = [b1_h0, b1_h1, b2_h0, b2_h1]
    d_b = nc.dram_tensor("d_b", [128, 4], f32, kind="ExternalInput").ap()
    d_s = nc.dram_tensor("d_s", [128, 2], f32, kind="ExternalOutput").ap()

    with tile.TileContext(nc) as tc, ExitStack() as ctx:
        cpool = ctx.enter_context(tc.tile_pool(name="cpool", bufs=1))
        xpool = ctx.enter_context(tc.tile_pool(name="xpool", bufs=3))
        hpool = ctx.enter_context(tc.tile_pool(name="hpool", bufs=2))
        spool = ctx.enter_context(tc.tile_pool(name="spool", bufs=1))
        ps1p = ctx.enter_context(tc.tile_pool(name="ps1p", bufs=1, space="PSUM"))
        ps2p = ctx.enter_context(tc.tile_pool(name="ps2p", bufs=1, space="PSUM"))

        w1_sb = cpool.tile([IN, H], f16, name="w1_sb")
        nc.sync.dma_start(w1_sb[:], d_w1)
        w2p_sb = cpool.tile([128, 2, H], fp8, name="w2p_sb")
        nc.sync.dma_start(w2p_sb[:], d_w2)
        bv = cpool.tile([128, 4], f32, name="bv")
        nc.sync.dma_start(bv[:], d_b)
        # negated b2_h1 for the DVE path: out = max(ps2, -b2) + b2, so the
        # accum reduce op (== op1) is add and accum_out is a true sum.
        nb2 = cpool.tile([128, 1], f32, name="nb2")
        nc.vector.tensor_scalar_mul(nb2[:], bv[:, 3:4], -1.0)

        acc = [cpool.tile([128, NPAIR], f32, name=f"acc{m}") for m in range(2)]

        for pair in [p for _ in range(iters) for p in range(NPAIR)]:
            xt = xpool.tile([IN, 2 * BLK], f16, name="xt", tag="xt")
            nc.sync.dma_start(xt[:], d_xt[:, pair * 2 * BLK : (pair + 1) * 2 * BLK])

            ps1 = [
                ps1p.tile([128, 2, 512], f32, name=f"ps1_{m}", tag=f"ps1_{m}")
                for m in range(2)
            ]
            for j in range(2):
                xr = xt[:, j * BLK : (j + 1) * BLK]
                for m in range(2):
                    nc.tensor.matmul(
                        ps1[m][:, j, 0:BLK],
                        w1_sb[:, m * 128 : (m + 1) * 128],
                        xr,
                        start=True,
                        stop=True,
                    )

            # (a): h1 = relu(ps1 + b1) -> packed fp8, one ACT op per half.
            h1 = hpool.tile([128, 2, 2, 512], fp8, name="h1", tag="h1")
            for m in range(2):
                nc.scalar.activation(
                    h1[:, m, :, 0:BLK],
                    ps1[m][:, :, 0:BLK],
                    Relu,
                    bias=bv[:, m : m + 1],
                )

            # Layer 2: DoubleRow, weights-outer so each half's LDWEIGHTS is
            # shared by both blocks of the pair.
            ps2 = [
                ps2p.tile([128, 2, 512], f32, name=f"ps2_{m}", tag=f"ps2_{m}")
                for m in range(2)
            ]
            for m in range(2):
                for j in range(2):
                    nc.tensor.matmul(
                        ps2[m][:, j, 0:BLK],
                        w2p_sb[:, :, m * 128 : (m + 1) * 128],
                        h1[:, :, j, 0:BLK],
                        start=True,
                        stop=True,
                        perf_mode=mybir.MatmulPerfMode.DoubleRow,
                    )

            # (b): relu(ps2 + b2) + per-channel row-sum; half0 on ACT,
            # half1 on DVE so the two drains run in parallel.
            scr0 = spool.tile([128, 2, 512], f16, name="scr0", tag="scr0")
            nc.scalar.activation(
                scr0[:, :, 0:BLK],
                ps2[0][:, :, 0:BLK],
                Relu,
                bias=bv[:, 2:3],
                accum_out=acc[0][:, pair : pair + 1],
            )
            scr1 = spool.tile([128, 2, 512], f16, name="scr1", tag="scr1")
            nc.vector.tensor_scalar(
                scr1[:, :, 0:BLK],
                ps2[1][:, :, 0:BLK],
                nb2[:],
                None,
                op0=Alu.max,
                op1=Alu.add,
                accum_out=acc[1][:, pair : pair + 1],
            )

        s_sb = cpool.tile([128, 2], f32, name="s_sb")
        for m in range(2):
            nc.vector.reduce_sum(s_sb[:, m : m + 1], acc[m][:], axis=X)
        nc.sync.dma_start(d_s, s_sb[:])

    nc.compile()
    return nc


def _build_v6(iters: int = 1):
    """Like v5 but layer-1 bias rides in the matmul contraction (K=65
    ones-row, as in the original baseline), so layer-1 relu needs no bias
    and collapses to ONE ACT op per pair over a single 4-bank psum tile
    with slot index (2*block + half):
      ACT: (a) relu(ps1)->fp8 h1 FD=2000,  (b)h0 relu+bias+accum FD=1000
      DVE: (b)h1 add-bias+max+accum FD=1000
      PE : 4x K=65 f16 layer-1 mm + 4x DoubleRow layer-2 mm per pair
    """
    import concourse.mybir as mybir
    import concourse.tile as tile
    from concourse import bacc
    from contextlib import ExitStack

    dt = mybir.dt
    f32 = dt.float32
    f16 = dt.float16
    fp8 = dt.float8e4
    Relu = mybir.ActivationFunctionType.Relu
    Alu = mybir.AluOpType
    X = mybir.AxisListType.X

    nc = bacc.Bacc(
        "TRN2",
        target_bir_lowering=False,
        debug=False,
        enable_asserts=False,
        num_devices=1,
    )

    d_xt = nc.dram_tensor("d_xt", [IN + 1, R], f16, kind="ExternalInput").ap()
    d_w1 = nc.dram_tensor("d_w1", [IN + 1, H], f16, kind="ExternalInput").ap()
    d_w2 = nc.dram_tensor("d_w2", [128, 2, H], fp8, kind="ExternalInput").ap()
    d_b2 = nc.dram_tensor("d_b2", [128, 2], f32, kind="ExternalInput").ap()
    d_s = nc.dram_tensor("d_s", [128, 2], f32, kind="ExternalOutput").ap()

    with tile.TileContext(nc) as tc, ExitStack() as ctx:
        cpool = ctx.enter_context(tc.tile_pool(name="cpool", bufs=1))
        xpool = ctx.enter_context(tc.tile_pool(name="xpool", bufs=3))
        hpool = ctx.enter_context(tc.tile_pool(name="hpool", bufs=2))
        spool = ctx.enter_context(tc.tile_pool(name="spool", bufs=1))
        ps1p = ctx.enter_context(tc.tile_pool(name="ps1p", bufs=1, space="PSUM"))
        ps2p = ctx.enter_context(tc.tile_pool(name="ps2p", bufs=1, space="PSUM"))

        w1_sb = cpool.tile([IN + 1, H], f16, name="w1_sb")
        nc.sync.dma_start(w1_sb[:], d_w1)
        w2p_sb = cpool.tile([128, 2, H], fp8, name="w2p_sb")
        nc.sync.dma_start(w2p_sb[:], d_w2)
        b2_sb = cpool.tile([128, 2], f32, name="b2_sb")
        nc.sync.dma_start(b2_sb[:], d_b2)
        # negated b2_h1 for the DVE path: out = max(ps2, -b2) + b2, so the
        # accum reduce op (== op1) is add and accum_out is a true sum.
        nb2 = cpool.tile([128, 1], f32, name="nb2")
        nc.vector.tensor_scalar_mul(nb2[:], b2_sb[:, 1:2], -1.0)

        acc = [cpool.tile([128, NPAIR], f32, name=f"acc{m}") for m in range(2)]

        for pair in [p for _ in range(iters) for p in range(NPAIR)]:
            xt = xpool.tile([IN + 1, 2 * BLK], f16, name="xt", tag="xt")
            nc.sync.dma_start(xt[:], d_xt[:, pair * 2 * BLK : (pair + 1) * 2 * BLK])

            # ps1 slots: index 2*block + half.
            ps1 = ps1p.tile([128, 4, 512], f32, name="ps1", tag="ps1")
            for j in range(2):
                xr = xt[:, j * BLK : (j + 1) * BLK]
                for m in range(2):
                    nc.tensor.matmul(
                        ps1[:, 2 * j + m, 0:BLK],
                        w1_sb[:, m * 128 : (m + 1) * 128],
                        xr,
                        start=True,
                        stop=True,
                    )

            # (a): ONE ACT op drains all four layer-1 banks -> packed fp8.
            h1 = hpool.tile([128, 4, 512], fp8, name="h1", tag="h1")
            nc.scalar.activation(h1[:, :, 0:BLK], ps1[:, :, 0:BLK], Relu)

            # Layer 2: DoubleRow, weights-outer; rhs pair-dim = half
            # (slots 2j..2j+1 are the two halves of block j).
            ps2 = [
                ps2p.tile([128, 2, 512], f32, name=f"ps2_{m}", tag=f"ps2_{m}")
                for m in range(2)
            ]
            for m in range(2):
                for j in range(2):
                    nc.tensor.matmul(
                        ps2[m][:, j, 0:BLK],
                        w2p_sb[:, :, m * 128 : (m + 1) * 128],
                        h1[:, 2 * j : 2 * j + 2, 0:BLK],
                        start=True,
                        stop=True,
                        perf_mode=mybir.MatmulPerfMode.DoubleRow,
                    )

            # (b): relu(ps2 + b2) + per-channel row-sum; half0 on ACT,
            # half1 on DVE.
            scr0 = spool.tile([128, 2, 512], f16, name="scr0", tag="scr0")
            nc.scalar.activation(
                scr0[:, :, 0:BLK],
                ps2[0][:, :, 0:BLK],
                Relu,
                bias=b2_sb[:, 0:1],
                accum_out=acc[0][:, pair : pair + 1],
            )
            scr1 = spool.tile([128, 2, 512], f16, name="scr1", tag="scr1")
            nc.vector.tensor_scalar(
                scr1[:, :, 0:BLK],
                ps2[1][:, :, 0:BLK],
                nb2[:],
                None,
                op0=Alu.max,
                op1=Alu.add,
                accum_out=acc[1][:, pair : pair + 1],
            )

        s_sb = cpool.tile([128, 2], f32, name="s_sb")
        for m in range(2):
            nc.vector.reduce_sum(s_sb[:, m : m + 1], acc[m][:], axis=X)
        nc.sync.dma_start(d_s, s_sb[:])

    nc.compile()
    return nc


def _diffuse_quant(W: np.ndarray, qdt) -> np.ndarray:
    """Error-diffusion quantization down the contraction axis: keeps
    per-column cumulative quantization error near zero so the (positive-mean)
    h1 stream doesn't see a systematic bias."""
    Wq = np.empty(W.shape, np.float32)
    carry = np.zeros(W.shape[1], np.float32)
    for k in range(W.shape[0]):
        t = W[k] + carry
        q = t.astype(qdt).astype(np.float32)
        carry = t - q
        Wq[k] = q
    return Wq


def _prep_in_maps(inputs: dict, mode: str):
    import ml_dtypes

    fp8 = mode in ("v3fp8", "v4", "v5", "v6")
    x = np.asarray(inputs["x"], dtype=np.float32)
    pw1 = np.asarray(inputs["pw1"], dtype=np.float16)
    pb1 = np.asarray(inputs["pb1"], dtype=np.float32)
    pw2 = np.asarray(inputs["pw2"], dtype=np.float32)
    pb2 = np.asarray(inputs["pb2"], dtype=np.float32)

    if fp8:
        w2q = _diffuse_quant(pw2, ml_dtypes.float8_e4m3)
        w2 = np.ascontiguousarray(
            w2q.reshape(2, 128, H).transpose(1, 0, 2)
        ).astype(ml_dtypes.float8_e4m3)  # [k, pair, m]
    else:
        w2 = pw2.astype(np.float16)

    common: dict
    aug = False
    if mode == "v4":
        b1m = np.zeros((128, 128), np.float16)
        b1m[64] = pb1[0:128].astype(np.float16)
        b1m[96] = pb1[128:256].astype(np.float16)
        b2m = np.stack([pb2[0:128], pb2[128:256]], axis=1).astype(np.float32)
        common = {"d_w1": pw1, "d_w2": w2, "d_b1": b1m, "d_b2": b2m}
    elif mode == "v6":
        aug = True
        w1a = np.concatenate([pw1, pb1[None, :].astype(np.float16)], axis=0)
        b2m = np.stack([pb2[0:128], pb2[128:256]], axis=1).astype(np.float32)
        common = {"d_w1": w1a, "d_w2": w2, "d_b2": b2m}
    else:
        b = np.stack(
            [pb1[0:128], pb1[128:256], pb2[0:128], pb2[128:256]], axis=1
        ).astype(np.float32)  # [128, 4]
        common = {"d_w1": pw1, "d_w2": w2, "d_b": b}

    in_maps = []
    for c in range(N_CORES):
        if aug:
            xt = np.empty((IN + 1, R), np.float16)
            xt[:IN] = x[c * R : (c + 1) * R].T.astype(np.float16)
            xt[IN] = 1.0
        else:
            xt = np.ascontiguousarray(x[c * R : (c + 1) * R].T).astype(np.float16)
        in_maps.append({"d_xt": xt, **common})
    return in_maps


def _host_tail(S: np.ndarray, inputs: dict) -> np.ndarray:
    f = np.float64

    def g(name):
        return np.asarray(inputs[name], dtype=f)

    phi_sum = S @ g("pw3") + N * g("pb3")
    r = np.maximum(phi_sum @ g("rw1") + g("rb1"), 0.0)
    r = np.maximum(r @ g("rw2") + g("rb2"), 0.0)
    r = r @ g("rw3") + g("rb3")
    v = np.concatenate([r, g("x_static")])
    v = np.maximum(v @ g("w1") + g("b1"), 0.0)
    v = np.maximum(v @ g("w2") + g("b2"), 0.0)
    return (v @ g("w3") + g("b3")).astype(np.float32)


def _run(inputs: dict, trace: bool = False, mode: str | None = None):
    from concourse.bass_utils import run_bass_kernel_spmd

    mode = mode or MODE
    nc = _prog_cache.get(mode)
    if nc is None:
        nc = _build(mode)
        _prog_cache[mode] = nc

    if trace:
        try:
            import antenv.axon_hooks  # noqa: F401
        except ImportError:
            trace = False

    in_maps = _prep_in_maps(inputs, mode)
    res = run_bass_kernel_spmd(
        nc,
        in_maps,
        core_ids=list(range(N_CORES)),
        trace=trace,
    )

    S = np.zeros(H, np.float64)
    for rmap in res.results:
        s = rmap["d_s"].astype(np.float64)  # [128, 2]; channel = m*128 + p
        S += s.T.reshape(H)
    if mode in ("v4", "v5", "v6"):
        # the DVE path for h2-half1 accumulates sum(max(ps2, -b2)); the
        # + b2 * row-count shift is exact and lands here.
        S[128:256] += N * np.asarray(inputs["pb2"], np.float64)[128:256]
    out = _host_tail(S, inputs)
    return out, res


def kernel(**inputs) -> np.ndarray:
    out, _ = _run(inputs)
    return out


# revision 24
# speedup vs baseline: 1.8909x; 1.8909x over previous
"""Trainium2 Bass kernel for nn_DQN_34136400069239 (DeepSets-style pooling).

Math (reference):
    h1  = relu(x @ pw1 + pb1)          [N, H]
    h2  = relu(h1 @ pw2 + pb2)         [N, H]
    phi = h2 @ pw3 + pb3               [N, F]
    fp  = sum(phi, axis=0)             [F]
    ... tiny rho MLP + concat(x_static) + tiny 3-layer MLP -> [OUT]

The third phi layer is linear, so fp = (sum_n h2[n]) @ pw3 + N * pb3 and the
device only computes S = sum_n relu(h2[n]) in R^H.  Data-parallel over rows:
8 cores x 50000 rows, host sums the 8 partial S vectors and runs the tail.

Device design (mode "v3*"), per 1000-row pair of 500-row blocks:
  - PSUM is laid out pair-level and half-major: ps1_h0/ps1_h1 and
    ps2_h0/ps2_h1 are [128, 2(block), 512] tiles.  Each vector-engine op
    then covers ONE h-half of TWO blocks, so its per-partition bias vector
    is uniform and accum_out keeps per-channel sums:
      DVE: h1 = max(ps1_h + b1_h, 0)   tensor_scalar(add, max), FD=1000
      ACT: relu(ps2_h + b2_h) + row-sum accum_out, FD=1000
    Those two engines are the roofline (~1167 / ~977 ns per 500-row block);
    biases ride along for free as exact-f32 per-partition operands.
  - mode "v3"    : layer 2 = 4 f16 matmuls (K=128) per block, PE ~3000 c/blk
  - mode "v3fp8" : layer 2 = 2 fp8e4m3 DoubleRow matmuls (K_eff=256) per
                   block, PE ~2100 c/blk; W2 quantized with error diffusion
                   down the contraction axis (plain fp8 rounding fails the
                   2e-2 gate at 2.3e-2; diffusion passes at ~3.5e-3).
"""

import os

import numpy as np

# Problem constants (hardcoded; kernel.py must be self-contained).
N = 400000
IN, H, F, S_STATIC, OUT = 64, 256, 128, 16, 5
N_CORES = 8
R = N // N_CORES  # rows per core = 50000
BLK = 500  # matmul moving free dim
NBLK = R // BLK  # 100
NPAIR = NBLK // 2  # 50

MODE = os.environ.get("DQN_MODE", "f16")

_prog_cache: dict = {}


def _build(mode: str, iters: int = 1):
    if mode == "v4":
        return _build_v4(iters)
    if mode == "v5":
        return _build_v5(iters)
    if mode == "v6":
        return _build_v6(iters)
    if mode == "v7":
        return _build_v7(iters)
    if mode in ("f16", "f32r", "f32r_split", "f32"):
        return _build_base(mode, iters)
    import concourse.mybir as mybir
    import concourse.tile as tile
    from concourse import bacc
    from contextlib import ExitStack

    dt = mybir.dt
    f32 = dt.float32
    f16 = dt.float16
    fp8 = mode == "v3fp8"
    h1_dt = dt.float8e4 if fp8 else f16

    nc = bacc.Bacc(
        "TRN2",
        target_bir_lowering=False,
        debug=False,
        enable_asserts=False,
        num_devices=1,
    )

    d_xt = nc.dram_tensor("d_xt", [IN, R], f16, kind="ExternalInput").ap()
    d_w1 = nc.dram_tensor("d_w1", [IN, H], f16, kind="ExternalInput").ap()
    if fp8:
        # packed [k, pair, m]: W2p[k, i, m] = W2q[128*i + k, m]
        d_w2 = nc.dram_tensor("d_w2", [128, 2, H], dt.float8e4, kind="ExternalInput").ap()
    else:
        d_w2 = nc.dram_tensor("d_w2", [H, H], f16, kind="ExternalInput").ap()
    # f32 per-partition biases: cols = [b1_h0, b1_h1, b2_h0, b2_h1]
    d_b = nc.dram_tensor("d_b", [128, 4], f32, kind="ExternalInput").ap()
    d_s = nc.dram_tensor("d_s", [128, 2], f32, kind="ExternalOutput").ap()

    Relu = mybir.ActivationFunctionType.Relu
    Alu = mybir.AluOpType
    X = mybir.AxisListType.X

    with tile.TileContext(nc) as tc, ExitStack() as ctx:
        cpool = ctx.enter_context(tc.tile_pool(name="cpool", bufs=1))
        xpool = ctx.enter_context(tc.tile_pool(name="xpool", bufs=3))
        hpool = ctx.enter_context(tc.tile_pool(name="hpool", bufs=2))
        spool = ctx.enter_context(tc.tile_pool(name="spool", bufs=1))
        ps1p = ctx.enter_context(tc.tile_pool(name="ps1p", bufs=1, space="PSUM"))
        ps2p = ctx.enter_context(tc.tile_pool(name="ps2p", bufs=1, space="PSUM"))

        # Constants resident in SBUF.
        w1_sb = cpool.tile([IN, H], f16, name="w1_sb")
        nc.sync.dma_start(w1_sb[:], d_w1)
        if fp8:
            w2p_sb = cpool.tile([128, 2, H], dt.float8e4, name="w2p_sb")
            nc.sync.dma_start(w2p_sb[:], d_w2)
        else:
            w2_sb = []
            for k in range(2):
                t = cpool.tile([128, H], f16, name=f"w2_sb{k}")
                nc.sync.dma_start(t[:], d_w2[k * 128 : (k + 1) * 128, :])
                w2_sb.append(t)
        bv = cpool.tile([128, 4], f32, name="bv")
        nc.sync.dma_start(bv[:], d_b)

        # Per-pair accumulated row-sums of relu(h2), one column per pair.
        acc = [cpool.tile([128, NPAIR], f32, name=f"acc{m}") for m in range(2)]

        for pair in [p for _ in range(iters) for p in range(NPAIR)]:
            xt = xpool.tile([IN, 2 * BLK], f16, name="xt", tag="xt")
            nc.sync.dma_start(xt[:], d_xt[:, pair * 2 * BLK : (pair + 1) * 2 * BLK])

            ps1 = [
                ps1p.tile([128, 2, 512], f32, name=f"ps1_{m}", tag=f"ps1_{m}")
                for m in range(2)
            ]
            ps2 = [
                ps2p.tile([128, 2, 512], f32, name=f"ps2_{m}", tag=f"ps2_{m}")
                for m in range(2)
            ]

            # Layer 1: 4 K=64 matmuls into half-major pair psum.
            for j in range(2):
                xr = xt[:, j * BLK : (j + 1) * BLK]
                for m in range(2):
                    nc.tensor.matmul(
                        ps1[m][:, j, 0:BLK],
                        w1_sb[:, m * 128 : (m + 1) * 128],
                        xr,
                        start=True,
                        stop=True,
                    )

            # h1 = relu(ps1 + b1): one DVE op per half (uniform bias vector).
            h1 = hpool.tile([128, 2, 2, 512], h1_dt, name="h1", tag="h1")
            for m in range(2):
                nc.vector.tensor_scalar(
                    h1[:, m, :, 0:BLK],
                    ps1[m][:, :, 0:BLK],
                    bv[:, m : m + 1],
                    0.0,
                    op0=Alu.add,
                    op1=Alu.max,
                )

            # Layer 2 into pair-level psum.
            for j in range(2):
                if fp8:
                    for m in range(2):
                        nc.tensor.matmul(
                            ps2[m][:, j, 0:BLK],
                            w2p_sb[:, :, m * 128 : (m + 1) * 128],
                            h1[:, :, j, 0:BLK],
                            start=True,
                            stop=True,
                            perf_mode=mybir.MatmulPerfMode.DoubleRow,
                        )
                else:
                    for m in range(2):
                        for k in range(2):
                            nc.tensor.matmul(
                                ps2[m][:, j, 0:BLK],
                                w2_sb[k][:, m * 128 : (m + 1) * 128],
                                h1[:, k, j, 0:BLK],
                                start=(k == 0),
                                stop=(k == 1),
                            )

            # relu(ps2 + b2) with fused row-sum; channels preserved because
            # each op spans one half of both blocks.
            for m in range(2):
                scr = spool.tile([128, 2, 512], f16, name=f"scr{m}", tag=f"scr{m}")
                nc.scalar.activation(
                    scr[:, :, 0:BLK],
                    ps2[m][:, :, 0:BLK],
                    Relu,
                    bias=bv[:, 2 + m : 3 + m],
                    accum_out=acc[m][:, pair : pair + 1],
                )

        s_sb = cpool.tile([128, 2], f32, name="s_sb")
        for m in range(2):
            nc.vector.reduce_sum(s_sb[:, m : m + 1], acc[m][:], axis=X)
        nc.sync.dma_start(d_s, s_sb[:])

    nc.compile()
    return nc


def _build_v4(iters: int = 1):
    """ACT-centric fp8 variant.

    Empirical per-op costs (probe.py, chained, psum-f32 src):
      ACT  = ~383 + 0.25*FD ns   (f16 out; 4x-packed stream)
      DVE  = ~397 + 0.71*FD ns
    so ACT is the cheap drain and op count is what matters.  Per 1000-row
    pair: ONE ACT op does relu(ps1) for all four [half,block] layer-1 banks
    (FD=2000; b1 pre-added by K=1 ones-matmuls on PE strips 2-3, concurrent
    with the K=64 layer-1 matmuls on strips 0-1); layer-2 relu+accum runs
    half0 on ACT, half1 on DVE (bias as per-partition vector operands).
    Layer 2 is 2 fp8 DoubleRow matmuls per block (K_eff=256), weights-outer
    so LDWEIGHTS amortizes over the pair.
    """
    import concourse.mybir as mybir
    import concourse.tile as tile
    from concourse import bacc
    from contextlib import ExitStack

    dt = mybir.dt
    f32 = dt.float32
    f16 = dt.float16
    fp8 = dt.float8e4
    Relu = mybir.ActivationFunctionType.Relu
    Alu = mybir.AluOpType
    X = mybir.AxisListType.X

    nc = bacc.Bacc(
        "TRN2",
        target_bir_lowering=False,
        debug=False,
        enable_asserts=False,
        num_devices=1,
    )

    d_xt = nc.dram_tensor("d_xt", [IN, R], f16, kind="ExternalInput").ap()
    d_w1 = nc.dram_tensor("d_w1", [IN, H], f16, kind="ExternalInput").ap()
    d_w2 = nc.dram_tensor("d_w2", [128, 2, H], fp8, kind="ExternalInput").ap()
    # f16 b1 halves for the ones-matmuls, rows 64/96; f32 b2 via vector ops.
    d_b1 = nc.dram_tensor("d_b1", [128, 128], f16, kind="ExternalInput").ap()
    d_b2 = nc.dram_tensor("d_b2", [128, 2], f32, kind="ExternalInput").ap()
    d_s = nc.dram_tensor("d_s", [128, 2], f32, kind="ExternalOutput").ap()

    with tile.TileContext(nc) as tc, ExitStack() as ctx:
        cpool = ctx.enter_context(tc.tile_pool(name="cpool", bufs=1))
        xpool = ctx.enter_context(tc.tile_pool(name="xpool", bufs=3))
        hpool = ctx.enter_context(tc.tile_pool(name="hpool", bufs=2))
        spool = ctx.enter_context(tc.tile_pool(name="spool", bufs=1))
        ps1p = ctx.enter_context(tc.tile_pool(name="ps1p", bufs=1, space="PSUM"))
        ps2p = ctx.enter_context(tc.tile_pool(name="ps2p", bufs=1, space="PSUM"))

        w1_sb = cpool.tile([IN, H], f16, name="w1_sb")
        nc.sync.dma_start(w1_sb[:], d_w1)
        w2p_sb = cpool.tile([128, 2, H], fp8, name="w2p_sb")
        nc.sync.dma_start(w2p_sb[:], d_w2)
        b1_sb = cpool.tile([128, 128], f16, name="b1_sb")
        nc.sync.dma_start(b1_sb[:], d_b1)
        b2_sb = cpool.tile([128, 2], f32, name="b2_sb")
        nc.sync.dma_start(b2_sb[:], d_b2)
        nb2 = cpool.tile([128, 1], f32, name="nb2")
        nc.vector.tensor_scalar_mul(nb2[:], b2_sb[:, 1:2], -1.0)
        ones_sb = cpool.tile([128, BLK], f16, name="ones_sb")
        nc.vector.memset(ones_sb[:], 1.0)

        acc = [cpool.tile([128, NPAIR], f32, name=f"acc{m}") for m in range(2)]

        for pair in [p for _ in range(iters) for p in range(NPAIR)]:
            xt = xpool.tile([IN, 2 * BLK], f16, name="xt", tag="xt")
            nc.sync.dma_start(xt[:], d_xt[:, pair * 2 * BLK : (pair + 1) * 2 * BLK])

            # ps1: [half, block] banks, 4 banks, one tile per pair.
            ps1 = ps1p.tile([128, 2, 2, 512], f32, name="ps1", tag="ps1")
            for j in range(2):
                xr = xt[:, j * BLK : (j + 1) * BLK]
                for m in range(2):
                    strip = 64 if m == 0 else 96
                    nc.tensor.matmul(
                        ps1[:, m, j, 0:BLK],
                        b1_sb[strip : strip + 1, 0:128],
                        ones_sb[strip : strip + 1, 0:BLK],
                        start=True,
                        stop=False,
                        tile_position=(strip, 0),
                        skip_group_check=True,
                    )
                    nc.tensor.matmul(
                        ps1[:, m, j, 0:BLK],
                        w1_sb[:, m * 128 : (m + 1) * 128],
                        xr,
                        start=False,
                        stop=True,
                        skip_group_check=True,
                    )

            # (a): one ACT op drains all of ps1 -> packed fp8 h1.
            h1 = hpool.tile([128, 2, 2, 512], fp8, name="h1", tag="h1")
            nc.scalar.activation(h1[:, :, :, 0:BLK], ps1[:, :, :, 0:BLK], Relu)

            # Layer 2: DoubleRow, weights-outer so each half's LDWEIGHTS is
            # shared by both blocks of the pair.
            ps2 = [
                ps2p.tile([128, 2, 512], f32, name=f"ps2_{m}", tag=f"ps2_{m}")
                for m in range(2)
            ]
            for m in range(2):
                for j in range(2):
                    nc.tensor.matmul(
                        ps2[m][:, j, 0:BLK],
                        w2p_sb[:, :, m * 128 : (m + 1) * 128],
                        h1[:, :, j, 0:BLK],
                        start=True,
                        stop=True,
                        perf_mode=mybir.MatmulPerfMode.DoubleRow,
                    )

            # (b): relu(ps2 + b2) + per-channel row-sum; half0 on ACT,
            # half1 on DVE so the two drains run in parallel.
            scr0 = spool.tile([128, 2, 512], f16, name="scr0", tag="scr0")
            nc.scalar.activation(
                scr0[:, :, 0:BLK],
                ps2[0][:, :, 0:BLK],
                Relu,
                bias=b2_sb[:, 0:1],
                accum_out=acc[0][:, pair : pair + 1],
            )
            scr1 = spool.tile([128, 2, 512], f16, name="scr1", tag="scr1")
            nc.vector.tensor_scalar(
                scr1[:, :, 0:BLK],
                ps2[1][:, :, 0:BLK],
                nb2[:],
                None,
                op0=Alu.max,
                op1=Alu.add,
                accum_out=acc[1][:, pair : pair + 1],
            )

        s_sb = cpool.tile([128, 2], f32, name="s_sb")
        for m in range(2):
            nc.vector.reduce_sum(s_sb[:, m : m + 1], acc[m][:], axis=X)
        nc.sync.dma_start(d_s, s_sb[:])

    nc.compile()
    return nc


def _build_v5(iters: int = 1):
    """fp8 DoubleRow layer 2 with probe-informed engine split.

    Empirical per-op costs (probe.py, chained, psum-f32 src, FD=1000):
      ACT relu+bias(+accum) ~633 ns ;  DVE 2-op(+accum) ~1267 ns
    Per 1000-row pair (ops all pair-level, half-major so the per-partition
    bias vector is uniform within each op):
      ACT: (a)h0, (a)h1  relu(ps1+b1)->fp8 h1,  (b)h0 relu+accum  ~1.9 us
      DVE: (b)h1 relu+accum                                       ~1.3 us
      PE : 4x K=64 f16 layer-1 mm + 4x DoubleRow K_eff=256 layer-2 mm
           (weights-outer so each half's LDWEIGHTS covers both blocks)
    """
    import concourse.mybir as mybir
    import concourse.tile as tile
    from concourse import bacc
    from contextlib import ExitStack

    dt = mybir.dt
    f32 = dt.float32
    f16 = dt.float16
    fp8 = dt.float8e4
    Relu = mybir.ActivationFunctionType.Relu
    Alu = mybir.AluOpType
    X = mybir.AxisListType.X

    nc = bacc.Bacc(
        "TRN2",
        target_bir_lowering=False,
        debug=False,
        enable_asserts=False,
        num_devices=1,
    )

    d_xt = nc.dram_tensor("d_xt", [IN, R], f16, kind="ExternalInput").ap()
    d_w1 = nc.dram_tensor("d_w1", [IN, H], f16, kind="ExternalInput").ap()
    d_w2 = nc.dram_tensor("d_w2", [128, 2, H], fp8, kind="ExternalInput").ap()
    # f32 per-partition biases: cols = [b1_h0, b1_h1, b2_h0, b2_h1]
    d_b = nc.dram_tensor("d_b", [128, 4], f32, kind="ExternalInput").ap()
    d_s = nc.dram_tensor("d_s", [128, 2], f32, kind="ExternalOutput").ap()

    with tile.TileContext(nc) as tc, ExitStack() as ctx:
        cpool = ctx.enter_context(tc.tile_pool(name="cpool", bufs=1))
        xpool = ctx.enter_context(tc.tile_pool(name="xpool", bufs=3))
        hpool = ctx.enter_context(tc.tile_pool(name="hpool", bufs=2))
        spool = ctx.enter_context(tc.tile_pool(name="spool", bufs=1))
        ps1p = ctx.enter_context(tc.tile_pool(name="ps1p", bufs=1, space="PSUM"))
        ps2p = ctx.enter_context(tc.tile_pool(name="ps2p", bufs=1, space="PSUM"))

        w1_sb = cpool.tile([IN, H], f16, name="w1_sb")
        nc.sync.dma_start(w1_sb[:], d_w1)
        w2p_sb = cpool.tile([128, 2, H], fp8, name="w2p_sb")
        nc.sync.dma_start(w2p_sb[:], d_w2)
        bv = cpool.tile([128, 4], f32, name="bv")
        nc.sync.dma_start(bv[:], d_b)
        # negated b2_h1 for the DVE path: out = max(ps2, -b2) + b2, so the
        # accum reduce op (== op1) is add and accum_out is a true sum.
        nb2 = cpool.tile([128, 1], f32, name="nb2")
        nc.vector.tensor_scalar_mul(nb2[:], bv[:, 3:4], -1.0)

        acc = [cpool.tile([128, NPAIR], f32, name=f"acc{m}") for m in range(2)]

        for pair in [p for _ in range(iters) for p in range(NPAIR)]:
            xt = xpool.tile([IN, 2 * BLK], f16, name="xt", tag="xt")
            nc.sync.dma_start(xt[:], d_xt[:, pair * 2 * BLK : (pair + 1) * 2 * BLK])

            ps1 = [
                ps1p.tile([128, 2, 512], f32, name=f"ps1_{m}", tag=f"ps1_{m}")
                for m in range(2)
            ]
            for j in range(2):
                xr = xt[:, j * BLK : (j + 1) * BLK]
                for m in range(2):
                    nc.tensor.matmul(
                        ps1[m][:, j, 0:BLK],
                        w1_sb[:, m * 128 : (m + 1) * 128],
                        xr,
                        start=True,
                        stop=True,
                    )

            # (a): h1 = relu(ps1 + b1) -> packed fp8, one ACT op per half.
            h1 = hpool.tile([128, 2, 2, 512], fp8, name="h1", tag="h1")
            for m in range(2):
                nc.scalar.activation(
                    h1[:, m, :, 0:BLK],
                    ps1[m][:, :, 0:BLK],
                    Relu,
                    bias=bv[:, m : m + 1],
                )

            # Layer 2: DoubleRow, weights-outer so each half's LDWEIGHTS is
            # shared by both blocks of the pair.
            ps2 = [
                ps2p.tile([128, 2, 512], f32, name=f"ps2_{m}", tag=f"ps2_{m}")
                for m in range(2)
            ]
            for m in range(2):
                for j in range(2):
                    nc.tensor.matmul(
                        ps2[m][:, j, 0:BLK],
                        w2p_sb[:, :, m * 128 : (m + 1) * 128],
                        h1[:, :, j, 0:BLK],
                        start=True,
                        stop=True,
                        perf_mode=mybir.MatmulPerfMode.DoubleRow,
                    )

            # (b): relu(ps2 + b2) + per-channel row-sum; half0 on ACT,
            # half1 on DVE so the two drains run in parallel.
            scr0 = spool.tile([128, 2, 512], f16, name="scr0", tag="scr0")
            nc.scalar.activation(
                scr0[:, :, 0:BLK],
                ps2[0][:, :, 0:BLK],
                Relu,
                bias=bv[:, 2:3],
                accum_out=acc[0][:, pair : pair + 1],
            )
            scr1 = spool.tile([128, 2, 512], f16, name="scr1", tag="scr1")
            nc.vector.tensor_scalar(
                scr1[:, :, 0:BLK],
                ps2[1][:, :, 0:BLK],
                nb2[:],
                None,
                op0=Alu.max,
                op1=Alu.add,
                accum_out=acc[1][:, pair : pair + 1],
            )

        s_sb = cpool.tile([128, 2], f32, name="s_sb")
        for m in range(2):
            nc.vector.reduce_sum(s_sb[:, m : m + 1], acc[m][:], axis=X)
        nc.sync.dma_start(d_s, s_sb[:])

    nc.compile()
    return nc


def _build_v6(iters: int = 1):
    """Like v5 but layer-1 bias rides in the matmul contraction (K=65
    ones-row, as in the original baseline), so layer-1 relu needs no bias
    and collapses to ONE ACT op per pair over a single 4-bank psum tile
    with slot index (2*block + half):
      ACT: (a) relu(ps1)->fp8 h1 FD=2000,  (b)h0 relu+bias+accum FD=1000
      DVE: (b)h1 add-bias+max+accum FD=1000
      PE : 4x K=65 f16 layer-1 mm + 4x DoubleRow layer-2 mm per pair
    """
    import concourse.mybir as mybir
    import concourse.tile as tile
    from concourse import bacc
    from contextlib import ExitStack

    dt = mybir.dt
    f32 = dt.float32
    f16 = dt.float16
    fp8 = dt.float8e4
    Relu = mybir.ActivationFunctionType.Relu
    Alu = mybir.AluOpType
    X = mybir.AxisListType.X

    nc = bacc.Bacc(
        "TRN2",
        target_bir_lowering=False,
        debug=False,
        enable_asserts=False,
        num_devices=1,
    )

    d_xt = nc.dram_tensor("d_xt", [IN + 1, R], f16, kind="ExternalInput").ap()
    d_w1 = nc.dram_tensor("d_w1", [IN + 1, H], f16, kind="ExternalInput").ap()
    d_w2 = nc.dram_tensor("d_w2", [128, 2, H], fp8, kind="ExternalInput").ap()
    d_b2 = nc.dram_tensor("d_b2", [128, 2], f32, kind="ExternalInput").ap()
    d_s = nc.dram_tensor("d_s", [128, 2], f32, kind="ExternalOutput").ap()

    with tile.TileContext(nc) as tc, ExitStack() as ctx:
        cpool = ctx.enter_context(tc.tile_pool(name="cpool", bufs=1))
        xpool = ctx.enter_context(tc.tile_pool(name="xpool", bufs=3))
        hpool = ctx.enter_context(tc.tile_pool(name="hpool", bufs=2))
        spool = ctx.enter_context(tc.tile_pool(name="spool", bufs=1))
        ps1p = ctx.enter_context(tc.tile_pool(name="ps1p", bufs=1, space="PSUM"))
        ps2p = ctx.enter_context(tc.tile_pool(name="ps2p", bufs=1, space="PSUM"))

        w1_sb = cpool.tile([IN + 1, H], f16, name="w1_sb")
        nc.sync.dma_start(w1_sb[:], d_w1)
        w2p_sb = cpool.tile([128, 2, H], fp8, name="w2p_sb")
        nc.sync.dma_start(w2p_sb[:], d_w2)
        b2_sb = cpool.tile([128, 2], f32, name="b2_sb")
        nc.sync.dma_start(b2_sb[:], d_b2)
        # negated b2_h1 for the DVE path: out = max(ps2, -b2) + b2, so the
        # accum reduce op (== op1) is add and accum_out is a true sum.
        nb2 = cpool.tile([128, 1], f32, name="nb2")
        nc.vector.tensor_scalar_mul(nb2[:], b2_sb[:, 1:2], -1.0)

        acc = [cpool.tile([128, NPAIR], f32, name=f"acc{m}") for m in range(2)]

        for pair in [p for _ in range(iters) for p in range(NPAIR)]:
            xt = xpool.tile([IN + 1, 2 * BLK], f16, name="xt", tag="xt")
            nc.sync.dma_start(xt[:], d_xt[:, pair * 2 * BLK : (pair + 1) * 2 * BLK])

            # ps1 slots: index 2*block + half.
            ps1 = ps1p.tile([128, 4, 512], f32, name="ps1", tag="ps1")
            for j in range(2):
                xr = xt[:, j * BLK : (j + 1) * BLK]
                for m in range(2):
                    nc.tensor.matmul(
                        ps1[:, 2 * j + m, 0:BLK],
                        w1_sb[:, m * 128 : (m + 1) * 128],
                        xr,
                        start=True,
                        stop=True,
                    )

            # (a): ONE ACT op drains all four layer-1 banks -> packed fp8.
            h1 = hpool.tile([128, 4, 512], fp8, name="h1", tag="h1")
            nc.scalar.activation(h1[:, :, 0:BLK], ps1[:, :, 0:BLK], Relu)

            # Layer 2: DoubleRow, weights-outer; rhs pair-dim = half
            # (slots 2j..2j+1 are the two halves of block j).
            ps2 = [
                ps2p.tile([128, 2, 512], f32, name=f"ps2_{m}", tag=f"ps2_{m}")
                for m in range(2)
            ]
            for m in range(2):
                for j in range(2):
                    nc.tensor.matmul(
                        ps2[m][:, j, 0:BLK],
                        w2p_sb[:, :, m * 128 : (m + 1) * 128],
                        h1[:, 2 * j : 2 * j + 2, 0:BLK],
                        start=True,
                        stop=True,
                        perf_mode=mybir.MatmulPerfMode.DoubleRow,
                    )

            # (b): relu(ps2 + b2) + per-channel row-sum; half0 on ACT,
            # half1 on DVE.
            scr0 = spool.tile([128, 2, 512], f16, name="scr0", tag="scr0")
            nc.scalar.activation(
                scr0[:, :, 0:BLK],
                ps2[0][:, :, 0:BLK],
                Relu,
                bias=b2_sb[:, 0:1],
                accum_out=acc[0][:, pair : pair + 1],
            )
            scr1 = spool.tile([128, 2, 512], f16, name="scr1", tag="scr1")
            nc.vector.tensor_scalar(
                scr1[:, :, 0:BLK],
                ps2[1][:, :, 0:BLK],
                nb2[:],
                None,
                op0=Alu.max,
                op1=Alu.add,
                accum_out=acc[1][:, pair : pair + 1],
            )

        s_sb = cpool.tile([128, 2], f32, name="s_sb")
        for m in range(2):
            nc.vector.reduce_sum(s_sb[:, m : m + 1], acc[m][:], axis=X)
        nc.sync.dma_start(d_s, s_sb[:])

    nc.compile()
    return nc


def _build_v7(iters: int = 1):
    """v5 with strip-concurrent layer 1.

    Layer-1 matmuls are K=64 and use only PE row strips 0-1, so two of them
    run CONCURRENTLY when placed at tile_position (0,0) and (64,0): the host
    interleaves the pair's two 500-row blocks across partition halves
    (block A features in partitions 0-63, block B in 64-127) and W1 is
    duplicated across both halves.  Each pass computes one h-half of block A
    alongside the other h-half of block B, so layer 1 costs ~1000 PE cycles
    per pair instead of 2000.  Everything else is v5: fp8 DoubleRow layer 2,
    (a) = 2 pair-level ACT relu+bias ops, (b) = ACT half0 / DVE half1 (DVE
    accumulates sum(max(ps2,-b2)); + b2*N lands in the host tail).
    """
    import concourse.mybir as mybir
    import concourse.tile as tile
    from concourse import bacc
    from contextlib import ExitStack

    dt = mybir.dt
    f32 = dt.float32
    f16 = dt.float16
    fp8 = dt.float8e4
    Relu = mybir.ActivationFunctionType.Relu
    Alu = mybir.AluOpType
    X = mybir.AxisListType.X

    R2 = R // 2  # columns of the interleaved x layout

    nc = bacc.Bacc(
        "TRN2",
        target_bir_lowering=False,
        debug=False,
        enable_asserts=False,
        num_devices=1,
    )

    d_xt = nc.dram_tensor("d_xt", [128, R2], f16, kind="ExternalInput").ap()
    d_w1 = nc.dram_tensor("d_w1", [128, H], f16, kind="ExternalInput").ap()
    d_w2 = nc.dram_tensor("d_w2", [128, 2, H], fp8, kind="ExternalInput").ap()
    d_b = nc.dram_tensor("d_b", [128, 4], f32, kind="ExternalInput").ap()
    d_s = nc.dram_tensor("d_s", [128, 2], f32, kind="ExternalOutput").ap()

    with tile.TileContext(nc) as tc, ExitStack() as ctx:
        cpool = ctx.enter_context(tc.tile_pool(name="cpool", bufs=1))
        xpool = ctx.enter_context(tc.tile_pool(name="xpool", bufs=3))
        hpool = ctx.enter_context(tc.tile_pool(name="hpool", bufs=2))
        spool = ctx.enter_context(tc.tile_pool(name="spool", bufs=1))
        ps1p = ctx.enter_context(tc.tile_pool(name="ps1p", bufs=1, space="PSUM"))
        ps2p = ctx.enter_context(tc.tile_pool(name="ps2p", bufs=1, space="PSUM"))

        # W1 duplicated across both partition halves: [128, 256] f16.
        w1_sb = cpool.tile([128, H], f16, name="w1_sb")
        nc.sync.dma_start(w1_sb[:], d_w1)
        w2p_sb = cpool.tile([128, 2, H], fp8, name="w2p_sb")
        nc.sync.dma_start(w2p_sb[:], d_w2)
        bv = cpool.tile([128, 4], f32, name="bv")
        nc.sync.dma_start(bv[:], d_b)
        nb2 = cpool.tile([128, 1], f32, name="nb2")
        nc.vector.tensor_scalar_mul(nb2[:], bv[:, 3:4], -1.0)

        acc = [cpool.tile([128, NPAIR], f32, name=f"acc{m}") for m in range(2)]

        for pair in [p for _ in range(iters) for p in range(NPAIR)]:
            # [128, 500]: partitions 0-63 = block A features, 64-127 = block B.
            xt = xpool.tile([128, BLK], f16, name="xt", tag="xt")
            nc.sync.dma_start(xt[:], d_xt[:, pair * BLK : (pair + 1) * BLK])

            ps1 = [
                ps1p.tile([128, 2, 512], f32, name=f"ps1_{m}", tag=f"ps1_{m}")
                for m in range(2)
            ]
            # Pass 1: A-h0 on strips 0-1 || B-h1 on strips 2-3.
            nc.tensor.matmul(
                ps1[0][:, 0, 0:BLK], w1_sb[0:64, 0:128], xt[0:64, 0:BLK],
                start=True, stop=True,
            )
            nc.tensor.matmul(
                ps1[1][:, 1, 0:BLK], w1_sb[64:128, 128:256], xt[64:128, 0:BLK],
                start=True, stop=True,
            )
            # Pass 2: A-h1 || B-h0.
            nc.tensor.matmul(
                ps1[1][:, 0, 0:BLK], w1_sb[0:64, 128:256], xt[0:64, 0:BLK],
                start=True, stop=True,
            )
            nc.tensor.matmul(
                ps1[0][:, 1, 0:BLK], w1_sb[64:128, 0:128], xt[64:128, 0:BLK],
                start=True, stop=True,
            )

            # (a): h1 = relu(ps1 + b1) -> packed fp8, one ACT op per half.
            h1 = hpool.tile([128, 2, 2, 512], fp8, name="h1", tag="h1")
            for m in range(2):
                nc.scalar.activation(
                    h1[:, m, :, 0:BLK],
                    ps1[m][:, :, 0:BLK],
                    Relu,
                    bias=bv[:, m : m + 1],
                )

            ps2 = [
                ps2p.tile([128, 2, 512], f32, name=f"ps2_{m}", tag=f"ps2_{m}")
                for m in range(2)
            ]
            for m in range(2):
                for j in range(2):
                    nc.tensor.matmul(
                        ps2[m][:, j, 0:BLK],
                        w2p_sb[:, :, m * 128 : (m + 1) * 128],
                        h1[:, :, j, 0:BLK],
                        start=True,
                        stop=True,
                        perf_mode=mybir.MatmulPerfMode.DoubleRow,
                    )

            scr0 = spool.tile([128, 2, 512], f16, name="scr0", tag="scr0")
            nc.scalar.activation(
                scr0[:, :, 0:BLK],
                ps2[0][:, :, 0:BLK],
                Relu,
                bias=bv[:, 2:3],
                accum_out=acc[0][:, pair : pair + 1],
            )
            scr1 = spool.tile([128, 2, 512], f16, name="scr1", tag="scr1")
            nc.vector.tensor_scalar(
                scr1[:, :, 0:BLK],
                ps2[1][:, :, 0:BLK],
                nb2[:],
                None,
                op0=Alu.max,
                op1=Alu.add,
                accum_out=acc[1][:, pair : pair + 1],
            )

        s_sb = cpool.tile([128, 2], f32, name="s_sb")
        for m in range(2):
            nc.vector.reduce_sum(s_sb[:, m : m + 1], acc[m][:], axis=X)
        nc.sync.dma_start(d_s, s_sb[:])

    nc.compile()
    return nc


def _build_base(mode: str, iters: int = 1, xbufs: int = 4, hbufs: int = 3):
    """The original staged baseline (f16 default): ones-row K=65 layer 1,
    f16 layer 2 in accumulation groups, DVE relu + 2 ACT relu+accum ops."""
    import concourse.mybir as mybir
    import concourse.tile as tile
    from concourse import bacc
    from contextlib import ExitStack

    dt = mybir.dt
    f32 = dt.float32
    split = mode == "f32r_split"
    mm_dt = {"f32r": dt.float32r, "f32r_split": dt.float32r, "f32": f32,
             "f16": dt.float16}[mode]

    nc = bacc.Bacc(
        "TRN2",
        target_bir_lowering=False,
        debug=False,
        enable_asserts=False,
        num_devices=1,
    )

    d_xt = nc.dram_tensor("d_xt", [IN + 1, R], mm_dt, kind="ExternalInput").ap()
    d_w1 = nc.dram_tensor("d_w1", [IN + 1, H], mm_dt, kind="ExternalInput").ap()
    d_w2 = nc.dram_tensor("d_w2", [H, H], mm_dt, kind="ExternalInput").ap()
    d_pb2 = nc.dram_tensor("d_pb2", [H], f32, kind="ExternalInput").ap()
    if split:
        d_w1l = nc.dram_tensor("d_w1l", [IN + 1, H], mm_dt, kind="ExternalInput").ap()
        d_w2l = nc.dram_tensor("d_w2l", [H, H], mm_dt, kind="ExternalInput").ap()
    d_s = nc.dram_tensor("d_s", [128, 2], f32, kind="ExternalOutput").ap()

    Relu = mybir.ActivationFunctionType.Relu
    X = mybir.AxisListType.X

    with tile.TileContext(nc) as tc, ExitStack() as ctx:
        cpool = ctx.enter_context(tc.tile_pool(name="cpool", bufs=1))
        xpool = ctx.enter_context(tc.tile_pool(name="xpool", bufs=xbufs))
        hpool = ctx.enter_context(tc.tile_pool(name="hpool", bufs=hbufs))
        spool = ctx.enter_context(tc.tile_pool(name="spool", bufs=2))
        ps1p = ctx.enter_context(tc.tile_pool(name="ps1p", bufs=2, space="PSUM"))
        ps2p = ctx.enter_context(tc.tile_pool(name="ps2p", bufs=2, space="PSUM"))

        w1_sb = cpool.tile([IN + 1, H], mm_dt, name="w1_sb")
        nc.sync.dma_start(w1_sb[:], d_w1)
        w2_sb = []
        for k in range(2):
            t = cpool.tile([128, H], mm_dt, name=f"w2_sb{k}")
            nc.sync.dma_start(t[:], d_w2[k * 128 : (k + 1) * 128, :])
            w2_sb.append(t)
        if split:
            w1l_sb = cpool.tile([IN + 1, H], mm_dt, name="w1l_sb")
            nc.sync.dma_start(w1l_sb[:], d_w1l)
            w2l_sb = []
            for k in range(2):
                t = cpool.tile([128, H], mm_dt, name=f"w2l_sb{k}")
                nc.sync.dma_start(t[:], d_w2l[k * 128 : (k + 1) * 128, :])
                w2l_sb.append(t)
        pb2_sb = cpool.tile([128, 2], f32, name="pb2_sb")
        nc.sync.dma_start(pb2_sb[:], d_pb2.rearrange("(m p) -> p m", p=128))

        acc = cpool.tile([128, 2, NBLK], f32, name="acc")

        for b in [b for _ in range(iters) for b in range(NBLK)]:
            xt = xpool.tile([IN + 1, BLK], mm_dt, name="xt", tag="xt")
            nc.sync.dma_start(xt[:], d_xt[:, b * BLK : (b + 1) * BLK])
            xr = xt[:]

            ps1 = ps1p.tile([128, 2, 512], f32, name="ps1", tag="ps1")
            for m in range(2):
                ms = slice(m * 128, (m + 1) * 128)
                nc.tensor.matmul(
                    ps1[:, m, 0:BLK], w1_sb[:, ms], xr,
                    start=True, stop=not split,
                )
                if split:
                    nc.tensor.matmul(
                        ps1[:, m, 0:BLK], w1l_sb[:, ms], xr,
                        start=False, stop=True,
                    )

            h1 = hpool.tile([128, 2, BLK], mm_dt, name="h1", tag="h1")
            nc.vector.tensor_scalar_max(h1[:], ps1[:, :, 0:BLK], 0.0)

            ps2 = ps2p.tile([128, 2, 512], f32, name="ps2", tag="ps2")
            for m in range(2):
                ms = slice(m * 128, (m + 1) * 128)
                mms = []
                for k in range(2):
                    mms.append((w2_sb[k][:, ms], h1[:, k, :]))
                    if split:
                        mms.append((w2l_sb[k][:, ms], h1[:, k, :]))
                for i, (lw, rr) in enumerate(mms):
                    nc.tensor.matmul(
                        ps2[:, m, 0:BLK], lw, rr,
                        start=(i == 0), stop=(i == len(mms) - 1),
                    )

            scr0 = spool.tile([128, BLK], f32, name="scr0", tag="scr0")
            nc.scalar.activation(
                scr0[:], ps2[:, 0, 0:BLK], Relu,
                bias=pb2_sb[:, 0:1],
                accum_out=acc[:, 0, b : b + 1],
            )
            scr1 = spool.tile([128, BLK], f32, name="scr1", tag="scr1")
            nc.scalar.activation(
                scr1[:], ps2[:, 1, 0:BLK], Relu,
                bias=pb2_sb[:, 1:2],
                accum_out=acc[:, 1, b : b + 1],
            )

        s_sb = cpool.tile([128, 2], f32, name="s_sb")
        nc.vector.reduce_sum(s_sb[:], acc[:], axis=X)
        nc.sync.dma_start(d_s, s_sb[:])

    nc.compile()
    return nc


def _hi_lo(w: np.ndarray):
    import ml_dtypes

    hi = np.asarray(w, dtype=ml_dtypes.bfloat16).astype(np.float32)
    lo = (w - hi).astype(np.float32)
    return hi, lo


def _diffuse_quant(W: np.ndarray, qdt) -> np.ndarray:
    """Error-diffusion quantization down the contraction axis: keeps
    per-column cumulative quantization error near zero so the (positive-mean)
    h1 stream doesn't see a systematic bias."""
    Wq = np.empty(W.shape, np.float32)
    carry = np.zeros(W.shape[1], np.float32)
    for k in range(W.shape[0]):
        t = W[k] + carry
        q = t.astype(qdt).astype(np.float32)
        carry = t - q
        Wq[k] = q
    return Wq


def _prep_in_maps(inputs: dict, mode: str):
    import ml_dtypes

    x = np.asarray(inputs["x"], dtype=np.float32)
    pw1 = np.asarray(inputs["pw1"], dtype=np.float32)
    pb1 = np.asarray(inputs["pb1"], dtype=np.float32)
    pw2 = np.asarray(inputs["pw2"], dtype=np.float32)
    pb2 = np.asarray(inputs["pb2"], dtype=np.float32)

    if mode in ("f16", "f32r", "f32r_split", "f32"):
        split = mode == "f32r_split"
        w1_aug = np.concatenate([pw1, pb1[None, :]], axis=0)  # [65, H]
        if split:
            w1h, w1l = _hi_lo(w1_aug)
            w2h, w2l = _hi_lo(pw2)
        else:
            w1h, w2h = w1_aug, pw2
        mm_np = np.float16 if mode == "f16" else np.float32
        w1h = w1h.astype(mm_np)
        w2h = w2h.astype(mm_np)
        in_maps = []
        for c in range(N_CORES):
            xt = np.empty((IN + 1, R), mm_np)
            xt[:IN] = x[c * R : (c + 1) * R].T.astype(mm_np)
            xt[IN] = 1.0
            m = {"d_xt": xt, "d_w1": w1h, "d_w2": w2h, "d_pb2": pb2}
            if split:
                m["d_w1l"] = w1l
                m["d_w2l"] = w2l
            in_maps.append(m)
        return in_maps

    fp8 = mode in ("v3fp8", "v4", "v5", "v6", "v7")
    if fp8:
        w2q = _diffuse_quant(pw2, ml_dtypes.float8_e4m3)
        w2 = np.ascontiguousarray(
            w2q.reshape(2, 128, H).transpose(1, 0, 2)
        ).astype(ml_dtypes.float8_e4m3)  # [k, pair, m]
    else:
        w2 = pw2.astype(np.float16)

    pw1h = pw1.astype(np.float16)
    common: dict
    if mode == "v4":
        b1m = np.zeros((128, 128), np.float16)
        b1m[64] = pb1[0:128].astype(np.float16)
        b1m[96] = pb1[128:256].astype(np.float16)
        b2m = np.stack([pb2[0:128], pb2[128:256]], axis=1).astype(np.float32)
        common = {"d_w1": pw1h, "d_w2": w2, "d_b1": b1m, "d_b2": b2m}
    elif mode == "v6":
        w1a = np.concatenate([pw1h, pb1[None, :].astype(np.float16)], axis=0)
        b2m = np.stack([pb2[0:128], pb2[128:256]], axis=1).astype(np.float32)
        common = {"d_w1": w1a, "d_w2": w2, "d_b2": b2m}
    elif mode == "v7":
        w1d = np.concatenate([pw1h, pw1h], axis=0)  # [128, 256]
        b = np.stack(
            [pb1[0:128], pb1[128:256], pb2[0:128], pb2[128:256]], axis=1
        ).astype(np.float32)
        common = {"d_w1": w1d, "d_w2": w2, "d_b": b}
    else:
        b = np.stack(
            [pb1[0:128], pb1[128:256], pb2[0:128], pb2[128:256]], axis=1
        ).astype(np.float32)  # [128, 4]
        common = {"d_w1": pw1h, "d_w2": w2, "d_b": b}

    in_maps = []
    for c in range(N_CORES):
        xc = x[c * R : (c + 1) * R].T.astype(np.float16)  # [64, R]
        if mode == "v6":
            xt = np.empty((IN + 1, R), np.float16)
            xt[:IN] = xc
            xt[IN] = 1.0
        elif mode == "v7":
            # interleave the pair's two 500-row blocks across partition
            # halves: [0:64] = even blocks, [64:128] = odd blocks.
            xr = xc.reshape(IN, NPAIR, 2, BLK)
            xt = np.concatenate(
                [
                    np.ascontiguousarray(xr[:, :, 0, :]).reshape(IN, R // 2),
                    np.ascontiguousarray(xr[:, :, 1, :]).reshape(IN, R // 2),
                ],
                axis=0,
            )  # [128, R//2]
        else:
            xt = np.ascontiguousarray(xc)
        in_maps.append({"d_xt": xt, **common})
    return in_maps


def _host_tail(S: np.ndarray, inputs: dict) -> np.ndarray:
    f = np.float64

    def g(name):
        return np.asarray(inputs[name], dtype=f)

    phi_sum = S @ g("pw3") + N * g("pb3")
    r = np.maximum(phi_sum @ g("rw1") + g("rb1"), 0.0)
    r = np.maximum(r @ g("rw2") + g("rb2"), 0.0)
    r = r @ g("rw3") + g("rb3")
    v = np.concatenate([r, g("x_static")])
    v = np.maximum(v @ g("w1") + g("b1"), 0.0)
    v = np.maximum(v @ g("w2") + g("b2"), 0.0)
    return (v @ g("w3") + g("b3")).astype(np.float32)


def _run(inputs: dict, trace: bool = False, mode: str | None = None):
    from concourse.bass_utils import run_bass_kernel_spmd

    mode = mode or MODE
    nc = _prog_cache.get(mode)
    if nc is None:
        nc = _build(mode)
        _prog_cache[mode] = nc

    if trace:
        try:
            import antenv.axon_hooks  # noqa: F401
        except ImportError:
            trace = False

    in_maps = _prep_in_maps(inputs, mode)
    res = run_bass_kernel_spmd(
        nc,
        in_maps,
        core_ids=list(range(N_CORES)),
        trace=trace,
    )

    S = np.zeros(H, np.float64)
    for rmap in res.results:
        s = rmap["d_s"].astype(np.float64)  # [128, 2]; channel = m*128 + p
        S += s.T.reshape(H)
    if mode in ("v4", "v5", "v6", "v7"):
        # the DVE path for h2-half1 accumulates sum(max(ps2, -b2)); the
        # + b2 * row-count shift is exact and lands here.
        S[128:256] += N * np.asarray(inputs["pb2"], np.float64)[128:256]
    out = _host_tail(S, inputs)
    return out, res


def kernel(**inputs) -> np.ndarray:
    out, _ = _run(inputs)
    return out


# revision 26
# speedup vs baseline: 1.9572x; 1.0351x over previous
"""Trainium2 Bass kernel for nn_DQN_34136400069239 (DeepSets-style pooling).

Math (reference):
    h1  = relu(x @ pw1 + pb1)          [N, H]
    h2  = relu(h1 @ pw2 + pb2)         [N, H]
    phi = h2 @ pw3 + pb3               [N, F]
    fp  = sum(phi, axis=0)             [F]
    ... tiny rho MLP + concat(x_static) + tiny 3-layer MLP -> [OUT]

The third phi layer is linear, so fp = (sum_n h2[n]) @ pw3 + N * pb3 and the
device only computes S = sum_n relu(h2[n]) in R^H.  Data-parallel over rows:
8 cores x 50000 rows, host sums the 8 partial S vectors and runs the tail.

Default mode "v7" (measured 141.3 us local-slope vs 176.1 us for the staged
f16 baseline; rel err 3.4e-3), per 1000-row pair of 500-row blocks:
  - Strip-concurrent layer 1: the K=64 matmuls use only half the PE rows,
    so the host interleaves the pair's two blocks across partition halves
    (block A features in partitions 0-63, B in 64-127, W1 duplicated) and
    each pass runs one h-half of A at tile_position (0,0) CONCURRENTLY with
    the other h-half of B at (64,0) -> layer 1 is ~1000 PE cycles/pair
    instead of 2000.
  - Layer 2: 2 fp8e4m3 DoubleRow matmuls per block (K_eff=256 in one pass),
    weights-outer so LDWEIGHTS amortizes over the pair.  W2 is quantized
    with error diffusion down the contraction axis (plain fp8 rounding
    fails the 2e-2 gate at 2.3e-2; diffusion passes at ~3.4e-3).
  - PSUM is pair-level and half-major, so every vector-engine op covers one
    h-half of both blocks with a uniform per-partition bias vector:
      ACT: (a) h1 = relu(ps1+b1) -> packed fp8, one op per half (~633 ns);
           (b)h0 relu(ps2+b2)+accum row-sum (~633 ns)
      DVE: (b)h1 = sum(max(ps2,-b2)) via tensor_scalar(max, add, accum_out)
           (~1.3 us; the + b2*N shift is exact and lands in the host tail —
           accum_out reduces with op1 and applies scalar2 only once).
Other modes: f16/f32r/f32r_split = the original staged baseline; v3/v3fp8,
v5, v6 = intermediate restructures kept for comparison (v4 is broken).
"""

import os

import numpy as np

# Problem constants (hardcoded; kernel.py must be self-contained).
N = 400000
IN, H, F, S_STATIC, OUT = 64, 256, 128, 16, 5
N_CORES = 8
R = N // N_CORES  # rows per core = 50000
BLK = 500  # matmul moving free dim
NBLK = R // BLK  # 100
NPAIR = NBLK // 2  # 50

MODE = os.environ.get("DQN_MODE", "v7")

_prog_cache: dict = {}


def _build(mode: str, iters: int = 1):
    if mode == "v4":
        return _build_v4(iters)
    if mode == "v5":
        return _build_v5(iters)
    if mode == "v6":
        return _build_v6(iters)
    if mode == "v7":
        return _build_v7(iters)
    if mode == "v8":
        return _build_v8(iters)
    if mode in ("f16", "f32r", "f32r_split", "f32"):
        return _build_base(mode, iters)
    import concourse.mybir as mybir
    import concourse.tile as tile
    from concourse import bacc
    from contextlib import ExitStack

    dt = mybir.dt
    f32 = dt.float32
    f16 = dt.float16
    fp8 = mode == "v3fp8"
    h1_dt = dt.float8e4 if fp8 else f16

    nc = bacc.Bacc(
        "TRN2",
        target_bir_lowering=False,
        debug=False,
        enable_asserts=False,
        num_devices=1,
    )

    d_xt = nc.dram_tensor("d_xt", [IN, R], f16, kind="ExternalInput").ap()
    d_w1 = nc.dram_tensor("d_w1", [IN, H], f16, kind="ExternalInput").ap()
    if fp8:
        # packed [k, pair, m]: W2p[k, i, m] = W2q[128*i + k, m]
        d_w2 = nc.dram_tensor("d_w2", [128, 2, H], dt.float8e4, kind="ExternalInput").ap()
    else:
        d_w2 = nc.dram_tensor("d_w2", [H, H], f16, kind="ExternalInput").ap()
    # f32 per-partition biases: cols = [b1_h0, b1_h1, b2_h0, b2_h1]
    d_b = nc.dram_tensor("d_b", [128, 4], f32, kind="ExternalInput").ap()
    d_s = nc.dram_tensor("d_s", [128, 2], f32, kind="ExternalOutput").ap()

    Relu = mybir.ActivationFunctionType.Relu
    Alu = mybir.AluOpType
    X = mybir.AxisListType.X

    with tile.TileContext(nc) as tc, ExitStack() as ctx:
        cpool = ctx.enter_context(tc.tile_pool(name="cpool", bufs=1))
        xpool = ctx.enter_context(tc.tile_pool(name="xpool", bufs=3))
        hpool = ctx.enter_context(tc.tile_pool(name="hpool", bufs=2))
        spool = ctx.enter_context(tc.tile_pool(name="spool", bufs=1))
        ps1p = ctx.enter_context(tc.tile_pool(name="ps1p", bufs=1, space="PSUM"))
        ps2p = ctx.enter_context(tc.tile_pool(name="ps2p", bufs=1, space="PSUM"))

        # Constants resident in SBUF.
        w1_sb = cpool.tile([IN, H], f16, name="w1_sb")
        nc.sync.dma_start(w1_sb[:], d_w1)
        if fp8:
            w2p_sb = cpool.tile([128, 2, H], dt.float8e4, name="w2p_sb")
            nc.sync.dma_start(w2p_sb[:], d_w2)
        else:
            w2_sb = []
            for k in range(2):
                t = cpool.tile([128, H], f16, name=f"w2_sb{k}")
                nc.sync.dma_start(t[:], d_w2[k * 128 : (k + 1) * 128, :])
                w2_sb.append(t)
        bv = cpool.tile([128, 4], f32, name="bv")
        nc.sync.dma_start(bv[:], d_b)

        # Per-pair accumulated row-sums of relu(h2), one column per pair.
        acc = [cpool.tile([128, NPAIR], f32, name=f"acc{m}") for m in range(2)]

        for pair in [p for _ in range(iters) for p in range(NPAIR)]:
            xt = xpool.tile([IN, 2 * BLK], f16, name="xt", tag="xt")
            nc.sync.dma_start(xt[:], d_xt[:, pair * 2 * BLK : (pair + 1) * 2 * BLK])

            ps1 = [
                ps1p.tile([128, 2, 512], f32, name=f"ps1_{m}", tag=f"ps1_{m}")
                for m in range(2)
            ]
            ps2 = [
                ps2p.tile([128, 2, 512], f32, name=f"ps2_{m}", tag=f"ps2_{m}")
                for m in range(2)
            ]

            # Layer 1: 4 K=64 matmuls into half-major pair psum.
            for j in range(2):
                xr = xt[:, j * BLK : (j + 1) * BLK]
                for m in range(2):
                    nc.tensor.matmul(
                        ps1[m][:, j, 0:BLK],
                        w1_sb[:, m * 128 : (m + 1) * 128],
                        xr,
                        start=True,
                        stop=True,
                    )

            # h1 = relu(ps1 + b1): one DVE op per half (uniform bias vector).
            h1 = hpool.tile([128, 2, 2, 512], h1_dt, name="h1", tag="h1")
            for m in range(2):
                nc.vector.tensor_scalar(
                    h1[:, m, :, 0:BLK],
                    ps1[m][:, :, 0:BLK],
                    bv[:, m : m + 1],
                    0.0,
                    op0=Alu.add,
                    op1=Alu.max,
                )

            # Layer 2 into pair-level psum.
            for j in range(2):
                if fp8:
                    for m in range(2):
                        nc.tensor.matmul(
                            ps2[m][:, j, 0:BLK],
                            w2p_sb[:, :, m * 128 : (m + 1) * 128],
                            h1[:, :, j, 0:BLK],
                            start=True,
                            stop=True,
                            perf_mode=mybir.MatmulPerfMode.DoubleRow,
                        )
                else:
                    for m in range(2):
                        for k in range(2):
                            nc.tensor.matmul(
                                ps2[m][:, j, 0:BLK],
                                w2_sb[k][:, m * 128 : (m + 1) * 128],
                                h1[:, k, j, 0:BLK],
                                start=(k == 0),
                                stop=(k == 1),
                            )

            # relu(ps2 + b2) with fused row-sum; channels preserved because
            # each op spans one half of both blocks.
            for m in range(2):
                scr = spool.tile([128, 2, 512], f16, name=f"scr{m}", tag=f"scr{m}")
                nc.scalar.activation(
                    scr[:, :, 0:BLK],
                    ps2[m][:, :, 0:BLK],
                    Relu,
                    bias=bv[:, 2 + m : 3 + m],
                    accum_out=acc[m][:, pair : pair + 1],
                )

        s_sb = cpool.tile([128, 2], f32, name="s_sb")
        for m in range(2):
            nc.vector.reduce_sum(s_sb[:, m : m + 1], acc[m][:], axis=X)
        nc.sync.dma_start(d_s, s_sb[:])

    nc.compile()
    return nc


def _build_v4(iters: int = 1):
    """ACT-centric fp8 variant.

    Empirical per-op costs (probe.py, chained, psum-f32 src):
      ACT  = ~383 + 0.25*FD ns   (f16 out; 4x-packed stream)
      DVE  = ~397 + 0.71*FD ns
    so ACT is the cheap drain and op count is what matters.  Per 1000-row
    pair: ONE ACT op does relu(ps1) for all four [half,block] layer-1 banks
    (FD=2000; b1 pre-added by K=1 ones-matmuls on PE strips 2-3, concurrent
    with the K=64 layer-1 matmuls on strips 0-1); layer-2 relu+accum runs
    half0 on ACT, half1 on DVE (bias as per-partition vector operands).
    Layer 2 is 2 fp8 DoubleRow matmuls per block (K_eff=256), weights-outer
    so LDWEIGHTS amortizes over the pair.
    """
    import concourse.mybir as mybir
    import concourse.tile as tile
    from concourse import bacc
    from contextlib import ExitStack

    dt = mybir.dt
    f32 = dt.float32
    f16 = dt.float16
    fp8 = dt.float8e4
    Relu = mybir.ActivationFunctionType.Relu
    Alu = mybir.AluOpType
    X = mybir.AxisListType.X

    nc = bacc.Bacc(
        "TRN2",
        target_bir_lowering=False,
        debug=False,
        enable_asserts=False,
        num_devices=1,
    )

    d_xt = nc.dram_tensor("d_xt", [IN, R], f16, kind="ExternalInput").ap()
    d_w1 = nc.dram_tensor("d_w1", [IN, H], f16, kind="ExternalInput").ap()
    d_w2 = nc.dram_tensor("d_w2", [128, 2, H], fp8, kind="ExternalInput").ap()
    # f16 b1 halves for the ones-matmuls, rows 64/96; f32 b2 via vector ops.
    d_b1 = nc.dram_tensor("d_b1", [128, 128], f16, kind="ExternalInput").ap()
    d_b2 = nc.dram_tensor("d_b2", [128, 2], f32, kind="ExternalInput").ap()
    d_s = nc.dram_tensor("d_s", [128, 2], f32, kind="ExternalOutput").ap()

    with tile.TileContext(nc) as tc, ExitStack() as ctx:
        cpool = ctx.enter_context(tc.tile_pool(name="cpool", bufs=1))
        xpool = ctx.enter_context(tc.tile_pool(name="xpool", bufs=3))
        hpool = ctx.enter_context(tc.tile_pool(name="hpool", bufs=2))
        spool = ctx.enter_context(tc.tile_pool(name="spool", bufs=1))
        ps1p = ctx.enter_context(tc.tile_pool(name="ps1p", bufs=1, space="PSUM"))
        ps2p = ctx.enter_context(tc.tile_pool(name="ps2p", bufs=1, space="PSUM"))

        w1_sb = cpool.tile([IN, H], f16, name="w1_sb")
        nc.sync.dma_start(w1_sb[:], d_w1)
        w2p_sb = cpool.tile([128, 2, H], fp8, name="w2p_sb")
        nc.sync.dma_start(w2p_sb[:], d_w2)
        b1_sb = cpool.tile([128, 128], f16, name="b1_sb")
        nc.sync.dma_start(b1_sb[:], d_b1)
        b2_sb = cpool.tile([128, 2], f32, name="b2_sb")
        nc.sync.dma_start(b2_sb[:], d_b2)
        nb2 = cpool.tile([128, 1], f32, name="nb2")
        nc.vector.tensor_scalar_mul(nb2[:], b2_sb[:, 1:2], -1.0)
        ones_sb = cpool.tile([128, BLK], f16, name="ones_sb")
        nc.vector.memset(ones_sb[:], 1.0)

        acc = [cpool.tile([128, NPAIR], f32, name=f"acc{m}") for m in range(2)]

        for pair in [p for _ in range(iters) for p in range(NPAIR)]:
            xt = xpool.tile([IN, 2 * BLK], f16, name="xt", tag="xt")
            nc.sync.dma_start(xt[:], d_xt[:, pair * 2 * BLK : (pair + 1) * 2 * BLK])

            # ps1: [half, block] banks, 4 banks, one tile per pair.
            ps1 = ps1p.tile([128, 2, 2, 512], f32, name="ps1", tag="ps1")
            for j in range(2):
                xr = xt[:, j * BLK : (j + 1) * BLK]
                for m in range(2):
                    strip = 64 if m == 0 else 96
                    nc.tensor.matmul(
                        ps1[:, m, j, 0:BLK],
                        b1_sb[strip : strip + 1, 0:128],
                        ones_sb[strip : strip + 1, 0:BLK],
                        start=True,
                        stop=False,
                        tile_position=(strip, 0),
                        skip_group_check=True,
                    )
                    nc.tensor.matmul(
                        ps1[:, m, j, 0:BLK],
                        w1_sb[:, m * 128 : (m + 1) * 128],
                        xr,
                        start=False,
                        stop=True,
                        skip_group_check=True,
                    )

            # (a): one ACT op drains all of ps1 -> packed fp8 h1.
            h1 = hpool.tile([128, 2, 2, 512], fp8, name="h1", tag="h1")
            nc.scalar.activation(h1[:, :, :, 0:BLK], ps1[:, :, :, 0:BLK], Relu)

            # Layer 2: DoubleRow, weights-outer so each half's LDWEIGHTS is
            # shared by both blocks of the pair.
            ps2 = [
                ps2p.tile([128, 2, 512], f32, name=f"ps2_{m}", tag=f"ps2_{m}")
                for m in range(2)
            ]
            for m in range(2):
                for j in range(2):
                    nc.tensor.matmul(
                        ps2[m][:, j, 0:BLK],
                        w2p_sb[:, :, m * 128 : (m + 1) * 128],
                        h1[:, :, j, 0:BLK],
                        start=True,
                        stop=True,
                        perf_mode=mybir.MatmulPerfMode.DoubleRow,
                    )

            # (b): relu(ps2 + b2) + per-channel row-sum; half0 on ACT,
            # half1 on DVE so the two drains run in parallel.
            scr0 = spool.tile([128, 2, 512], f16, name="scr0", tag="scr0")
            nc.scalar.activation(
                scr0[:, :, 0:BLK],
                ps2[0][:, :, 0:BLK],
                Relu,
                bias=b2_sb[:, 0:1],
                accum_out=acc[0][:, pair : pair + 1],
            )
            scr1 = spool.tile([128, 2, 512], f16, name="scr1", tag="scr1")
            nc.vector.tensor_scalar(
                scr1[:, :, 0:BLK],
                ps2[1][:, :, 0:BLK],
                nb2[:],
                None,
                op0=Alu.max,
                op1=Alu.add,
                accum_out=acc[1][:, pair : pair + 1],
            )

        s_sb = cpool.tile([128, 2], f32, name="s_sb")
        for m in range(2):
            nc.vector.reduce_sum(s_sb[:, m : m + 1], acc[m][:], axis=X)
        nc.sync.dma_start(d_s, s_sb[:])

    nc.compile()
    return nc


def _build_v5(iters: int = 1):
    """fp8 DoubleRow layer 2 with probe-informed engine split.

    Empirical per-op costs (probe.py, chained, psum-f32 src, FD=1000):
      ACT relu+bias(+accum) ~633 ns ;  DVE 2-op(+accum) ~1267 ns
    Per 1000-row pair (ops all pair-level, half-major so the per-partition
    bias vector is uniform within each op):
      ACT: (a)h0, (a)h1  relu(ps1+b1)->fp8 h1,  (b)h0 relu+accum  ~1.9 us
      DVE: (b)h1 relu+accum                                       ~1.3 us
      PE : 4x K=64 f16 layer-1 mm + 4x DoubleRow K_eff=256 layer-2 mm
           (weights-outer so each half's LDWEIGHTS covers both blocks)
    """
    import concourse.mybir as mybir
    import concourse.tile as tile
    from concourse import bacc
    from contextlib import ExitStack

    dt = mybir.dt
    f32 = dt.float32
    f16 = dt.float16
    fp8 = dt.float8e4
    Relu = mybir.ActivationFunctionType.Relu
    Alu = mybir.AluOpType
    X = mybir.AxisListType.X

    nc = bacc.Bacc(
        "TRN2",
        target_bir_lowering=False,
        debug=False,
        enable_asserts=False,
        num_devices=1,
    )

    d_xt = nc.dram_tensor("d_xt", [IN, R], f16, kind="ExternalInput").ap()
    d_w1 = nc.dram_tensor("d_w1", [IN, H], f16, kind="ExternalInput").ap()
    d_w2 = nc.dram_tensor("d_w2", [128, 2, H], fp8, kind="ExternalInput").ap()
    # f32 per-partition biases: cols = [b1_h0, b1_h1, b2_h0, b2_h1]
    d_b = nc.dram_tensor("d_b", [128, 4], f32, kind="ExternalInput").ap()
    d_s = nc.dram_tensor("d_s", [128, 2], f32, kind="ExternalOutput").ap()

    with tile.TileContext(nc) as tc, ExitStack() as ctx:
        cpool = ctx.enter_context(tc.tile_pool(name="cpool", bufs=1))
        xpool = ctx.enter_context(tc.tile_pool(name="xpool", bufs=3))
        hpool = ctx.enter_context(tc.tile_pool(name="hpool", bufs=2))
        spool = ctx.enter_context(tc.tile_pool(name="spool", bufs=1))
        ps1p = ctx.enter_context(tc.tile_pool(name="ps1p", bufs=1, space="PSUM"))
        ps2p = ctx.enter_context(tc.tile_pool(name="ps2p", bufs=1, space="PSUM"))

        w1_sb = cpool.tile([IN, H], f16, name="w1_sb")
        nc.sync.dma_start(w1_sb[:], d_w1)
        w2p_sb = cpool.tile([128, 2, H], fp8, name="w2p_sb")
        nc.sync.dma_start(w2p_sb[:], d_w2)
        bv = cpool.tile([128, 4], f32, name="bv")
        nc.sync.dma_start(bv[:], d_b)
        # negated b2_h1 for the DVE path: out = max(ps2, -b2) + b2, so the
        # accum reduce op (== op1) is add and accum_out is a true sum.
        nb2 = cpool.tile([128, 1], f32, name="nb2")
        nc.vector.tensor_scalar_mul(nb2[:], bv[:, 3:4], -1.0)

        acc = [cpool.tile([128, NPAIR], f32, name=f"acc{m}") for m in range(2)]

        for pair in [p for _ in range(iters) for p in range(NPAIR)]:
            xt = xpool.tile([IN, 2 * BLK], f16, name="xt", tag="xt")
            nc.sync.dma_start(xt[:], d_xt[:, pair * 2 * BLK : (pair + 1) * 2 * BLK])

            ps1 = [
                ps1p.tile([128, 2, 512], f32, name=f"ps1_{m}", tag=f"ps1_{m}")
                for m in range(2)
            ]
            for j in range(2):
                xr = xt[:, j * BLK : (j + 1) * BLK]
                for m in range(2):
                    nc.tensor.matmul(
                        ps1[m][:, j, 0:BLK],
                        w1_sb[:, m * 128 : (m + 1) * 128],
                        xr,
                        start=True,
                        stop=True,
                    )

            # (a): h1 = relu(ps1 + b1) -> packed fp8, one ACT op per half.
            h1 = hpool.tile([128, 2, 2, 512], fp8, name="h1", tag="h1")
            for m in range(2):
                nc.scalar.activation(
                    h1[:, m, :, 0:BLK],
                    ps1[m][:, :, 0:BLK],
                    Relu,
                    bias=bv[:, m : m + 1],
                )

            # Layer 2: DoubleRow, weights-outer so each half's LDWEIGHTS is
            # shared by both blocks of the pair.
            ps2 = [
                ps2p.tile([128, 2, 512], f32, name=f"ps2_{m}", tag=f"ps2_{m}")
                for m in range(2)
            ]
            for m in range(2):
                for j in range(2):
                    nc.tensor.matmul(
                        ps2[m][:, j, 0:BLK],
                        w2p_sb[:, :, m * 128 : (m + 1) * 128],
                        h1[:, :, j, 0:BLK],
                        start=True,
                        stop=True,
                        perf_mode=mybir.MatmulPerfMode.DoubleRow,
                    )

            # (b): relu(ps2 + b2) + per-channel row-sum; half0 on ACT,
            # half1 on DVE so the two drains run in parallel.
            scr0 = spool.tile([128, 2, 512], f16, name="scr0", tag="scr0")
            nc.scalar.activation(
                scr0[:, :, 0:BLK],
                ps2[0][:, :, 0:BLK],
                Relu,
                bias=bv[:, 2:3],
                accum_out=acc[0][:, pair : pair + 1],
            )
            scr1 = spool.tile([128, 2, 512], f16, name="scr1", tag="scr1")
            nc.vector.tensor_scalar(
                scr1[:, :, 0:BLK],
                ps2[1][:, :, 0:BLK],
                nb2[:],
                None,
                op0=Alu.max,
                op1=Alu.add,
                accum_out=acc[1][:, pair : pair + 1],
            )

        s_sb = cpool.tile([128, 2], f32, name="s_sb")
        for m in range(2):
            nc.vector.reduce_sum(s_sb[:, m : m + 1], acc[m][:], axis=X)
        nc.sync.dma_start(d_s, s_sb[:])

    nc.compile()
    return nc


def _build_v6(iters: int = 1):
    """Like v5 but layer-1 bias rides in the matmul contraction (K=65
    ones-row, as in the original baseline), so layer-1 relu needs no bias
    and collapses to ONE ACT op per pair over a single 4-bank psum tile
    with slot index (2*block + half):
      ACT: (a) relu(ps1)->fp8 h1 FD=2000,  (b)h0 relu+bias+accum FD=1000
      DVE: (b)h1 add-bias+max+accum FD=1000
      PE : 4x K=65 f16 layer-1 mm + 4x DoubleRow layer-2 mm per pair
    """
    import concourse.mybir as mybir
    import concourse.tile as tile
    from concourse import bacc
    from contextlib import ExitStack

    dt = mybir.dt
    f32 = dt.float32
    f16 = dt.float16
    fp8 = dt.float8e4
    Relu = mybir.ActivationFunctionType.Relu
    Alu = mybir.AluOpType
    X = mybir.AxisListType.X

    nc = bacc.Bacc(
        "TRN2",
        target_bir_lowering=False,
        debug=False,
        enable_asserts=False,
        num_devices=1,
    )

    d_xt = nc.dram_tensor("d_xt", [IN + 1, R], f16, kind="ExternalInput").ap()
    d_w1 = nc.dram_tensor("d_w1", [IN + 1, H], f16, kind="ExternalInput").ap()
    d_w2 = nc.dram_tensor("d_w2", [128, 2, H], fp8, kind="ExternalInput").ap()
    d_b2 = nc.dram_tensor("d_b2", [128, 2], f32, kind="ExternalInput").ap()
    d_s = nc.dram_tensor("d_s", [128, 2], f32, kind="ExternalOutput").ap()

    with tile.TileContext(nc) as tc, ExitStack() as ctx:
        cpool = ctx.enter_context(tc.tile_pool(name="cpool", bufs=1))
        xpool = ctx.enter_context(tc.tile_pool(name="xpool", bufs=3))
        hpool = ctx.enter_context(tc.tile_pool(name="hpool", bufs=2))
        spool = ctx.enter_context(tc.tile_pool(name="spool", bufs=1))
        ps1p = ctx.enter_context(tc.tile_pool(name="ps1p", bufs=1, space="PSUM"))
        ps2p = ctx.enter_context(tc.tile_pool(name="ps2p", bufs=1, space="PSUM"))

        w1_sb = cpool.tile([IN + 1, H], f16, name="w1_sb")
        nc.sync.dma_start(w1_sb[:], d_w1)
        w2p_sb = cpool.tile([128, 2, H], fp8, name="w2p_sb")
        nc.sync.dma_start(w2p_sb[:], d_w2)
        b2_sb = cpool.tile([128, 2], f32, name="b2_sb")
        nc.sync.dma_start(b2_sb[:], d_b2)
        # negated b2_h1 for the DVE path: out = max(ps2, -b2) + b2, so the
        # accum reduce op (== op1) is add and accum_out is a true sum.
        nb2 = cpool.tile([128, 1], f32, name="nb2")
        nc.vector.tensor_scalar_mul(nb2[:], b2_sb[:, 1:2], -1.0)

        acc = [cpool.tile([128, NPAIR], f32, name=f"acc{m}") for m in range(2)]

        for pair in [p for _ in range(iters) for p in range(NPAIR)]:
            xt = xpool.tile([IN + 1, 2 * BLK], f16, name="xt", tag="xt")
            nc.sync.dma_start(xt[:], d_xt[:, pair * 2 * BLK : (pair + 1) * 2 * BLK])

            # ps1 slots: index 2*block + half.
            ps1 = ps1p.tile([128, 4, 512], f32, name="ps1", tag="ps1")
            for j in range(2):
                xr = xt[:, j * BLK : (j + 1) * BLK]
                for m in range(2):
                    nc.tensor.matmul(
                        ps1[:, 2 * j + m, 0:BLK],
                        w1_sb[:, m * 128 : (m + 1) * 128],
                        xr,
                        start=True,
                        stop=True,
                    )

            # (a): ONE ACT op drains all four layer-1 banks -> packed fp8.
            h1 = hpool.tile([128, 4, 512], fp8, name="h1", tag="h1")
            nc.scalar.activation(h1[:, :, 0:BLK], ps1[:, :, 0:BLK], Relu)

            # Layer 2: DoubleRow, weights-outer; rhs pair-dim = half
            # (slots 2j..2j+1 are the two halves of block j).
            ps2 = [
                ps2p.tile([128, 2, 512], f32, name=f"ps2_{m}", tag=f"ps2_{m}")
                for m in range(2)
            ]
            for m in range(2):
                for j in range(2):
                    nc.tensor.matmul(
                        ps2[m][:, j, 0:BLK],
                        w2p_sb[:, :, m * 128 : (m + 1) * 128],
                        h1[:, 2 * j : 2 * j + 2, 0:BLK],
                        start=True,
                        stop=True,
                        perf_mode=mybir.MatmulPerfMode.DoubleRow,
                    )

            # (b): relu(ps2 + b2) + per-channel row-sum; half0 on ACT,
            # half1 on DVE.
            scr0 = spool.tile([128, 2, 512], f16, name="scr0", tag="scr0")
            nc.scalar.activation(
                scr0[:, :, 0:BLK],
                ps2[0][:, :, 0:BLK],
                Relu,
                bias=b2_sb[:, 0:1],
                accum_out=acc[0][:, pair : pair + 1],
            )
            scr1 = spool.tile([128, 2, 512], f16, name="scr1", tag="scr1")
            nc.vector.tensor_scalar(
                scr1[:, :, 0:BLK],
                ps2[1][:, :, 0:BLK],
                nb2[:],
                None,
                op0=Alu.max,
                op1=Alu.add,
                accum_out=acc[1][:, pair : pair + 1],
            )

        s_sb = cpool.tile([128, 2], f32, name="s_sb")
        for m in range(2):
            nc.vector.reduce_sum(s_sb[:, m : m + 1], acc[m][:], axis=X)
        nc.sync.dma_start(d_s, s_sb[:])

    nc.compile()
    return nc


def _build_v8(iters: int = 1):
    """v5 with strip-concurrent layer 1.

    Layer-1 matmuls are K=64 and use only PE row strips 0-1, so two of them
    run CONCURRENTLY when placed at tile_position (0,0) and (64,0): the host
    interleaves the pair's two 500-row blocks across partition halves
    (block A features in partitions 0-63, block B in 64-127) and W1 is
    duplicated across both halves.  Each pass computes one h-half of block A
    alongside the other h-half of block B, so layer 1 costs ~1000 PE cycles
    per pair instead of 2000.  Everything else is v5: fp8 DoubleRow layer 2,
    (a) = 2 pair-level ACT relu+bias ops, (b) = ACT half0 / DVE half1 (DVE
    accumulates sum(max(ps2,-b2)); + b2*N lands in the host tail).
    """
    import concourse.mybir as mybir
    import concourse.tile as tile
    from concourse import bacc
    from contextlib import ExitStack

    dt = mybir.dt
    f32 = dt.float32
    f16 = dt.float16
    fp8 = dt.float8e4
    Relu = mybir.ActivationFunctionType.Relu
    Alu = mybir.AluOpType
    X = mybir.AxisListType.X

    R2 = R // 2  # columns of the interleaved x layout

    nc = bacc.Bacc(
        "TRN2",
        target_bir_lowering=False,
        debug=False,
        enable_asserts=False,
        num_devices=1,
    )

    d_xt = nc.dram_tensor("d_xt", [128, R2], f16, kind="ExternalInput").ap()
    d_w1 = nc.dram_tensor("d_w1", [128, H], f16, kind="ExternalInput").ap()
    d_w2 = nc.dram_tensor("d_w2", [128, 2, H], fp8, kind="ExternalInput").ap()
    d_b = nc.dram_tensor("d_b", [128, 4], f32, kind="ExternalInput").ap()
    d_s = nc.dram_tensor("d_s", [128, 2], f32, kind="ExternalOutput").ap()

    with tile.TileContext(nc) as tc, ExitStack() as ctx:
        cpool = ctx.enter_context(tc.tile_pool(name="cpool", bufs=1))
        xpool = ctx.enter_context(tc.tile_pool(name="xpool", bufs=3))
        hpool = ctx.enter_context(tc.tile_pool(name="hpool", bufs=3))
        spool = ctx.enter_context(tc.tile_pool(name="spool", bufs=1))
        ps1p = ctx.enter_context(tc.tile_pool(name="ps1p", bufs=1, space="PSUM"))
        ps2p = ctx.enter_context(tc.tile_pool(name="ps2p", bufs=1, space="PSUM"))

        # W1 duplicated across both partition halves: [128, 256] f16.
        w1_sb = cpool.tile([128, H], f16, name="w1_sb")
        nc.sync.dma_start(w1_sb[:], d_w1)
        w2p_sb = cpool.tile([128, 2, H], fp8, name="w2p_sb")
        nc.sync.dma_start(w2p_sb[:], d_w2)
        bv = cpool.tile([128, 4], f32, name="bv")
        nc.sync.dma_start(bv[:], d_b)
        nb2 = cpool.tile([128, 1], f32, name="nb2")
        nc.vector.tensor_scalar_mul(nb2[:], bv[:, 3:4], -1.0)

        acc = [cpool.tile([128, NPAIR], f32, name=f"acc{m}") for m in range(2)]

        for pair in [p for _ in range(iters) for p in range(NPAIR)]:
            # [128, 500]: partitions 0-63 = block A features, 64-127 = block B.
            xt = xpool.tile([128, BLK], f16, name="xt", tag="xt")
            nc.sync.dma_start(xt[:], d_xt[:, pair * BLK : (pair + 1) * BLK])

            ps1 = [
                ps1p.tile([128, 2, 512], f32, name=f"ps1_{m}", tag=f"ps1_{m}")
                for m in range(2)
            ]
            # Pass 1: A-h0 on strips 0-1 || B-h1 on strips 2-3.
            nc.tensor.matmul(
                ps1[0][:, 0, 0:BLK], w1_sb[0:64, 0:128], xt[0:64, 0:BLK],
                start=True, stop=True,
            )
            nc.tensor.matmul(
                ps1[1][:, 1, 0:BLK], w1_sb[64:128, 128:256], xt[64:128, 0:BLK],
                start=True, stop=True,
            )
            # Pass 2: A-h1 || B-h0.
            nc.tensor.matmul(
                ps1[1][:, 0, 0:BLK], w1_sb[0:64, 128:256], xt[0:64, 0:BLK],
                start=True, stop=True,
            )
            nc.tensor.matmul(
                ps1[0][:, 1, 0:BLK], w1_sb[64:128, 0:128], xt[64:128, 0:BLK],
                start=True, stop=True,
            )

            # (a): h1 = relu(ps1 + b1) -> packed fp8, one ACT op per half.
            h1 = hpool.tile([128, 2, 2, 512], fp8, name="h1", tag="h1")
            for m in range(2):
                nc.scalar.activation(
                    h1[:, m, :, 0:BLK],
                    ps1[m][:, :, 0:BLK],
                    Relu,
                    bias=bv[:, m : m + 1],
                )

            ps2 = [
                ps2p.tile([128, 2, 512], f32, name=f"ps2_{m}", tag=f"ps2_{m}")
                for m in range(2)
            ]
            for j in range(2):
                for m in range(2):
                    nc.tensor.matmul(
                        ps2[m][:, j, 0:BLK],
                        w2p_sb[:, :, m * 128 : (m + 1) * 128],
                        h1[:, :, j, 0:BLK],
                        start=True,
                        stop=True,
                        perf_mode=mybir.MatmulPerfMode.DoubleRow,
                    )
            # keep-warm blips: tiny matmuls into psum padding keep PE
            # activity in every HAM window (~50 ns each, no readers).
            nc.tensor.matmul(
                ps1[0][0:8, 0, 500:512], w1_sb[0:1, 0:8], w1_sb[0:1, 0:12],
                start=True, stop=True,
            )
            nc.tensor.matmul(
                ps1[1][0:8, 0, 500:512], w1_sb[0:1, 0:8], w1_sb[0:1, 0:12],
                start=True, stop=True,
            )

            scr0 = spool.tile([128, 2, 512], f16, name="scr0", tag="scr0")
            nc.scalar.activation(
                scr0[:, :, 0:BLK],
                ps2[0][:, :, 0:BLK],
                Relu,
                bias=bv[:, 2:3],
                accum_out=acc[0][:, pair : pair + 1],
            )
            scr1 = spool.tile([128, 2, 512], f16, name="scr1", tag="scr1")
            nc.vector.tensor_scalar(
                scr1[:, :, 0:BLK],
                ps2[1][:, :, 0:BLK],
                nb2[:],
                None,
                op0=Alu.max,
                op1=Alu.add,
                accum_out=acc[1][:, pair : pair + 1],
            )

        s_sb = cpool.tile([128, 2], f32, name="s_sb")
        for m in range(2):
            nc.vector.reduce_sum(s_sb[:, m : m + 1], acc[m][:], axis=X)
        nc.sync.dma_start(d_s, s_sb[:])

    nc.compile()
    return nc


def _build_v7(iters: int = 1):
    """v5 with strip-concurrent layer 1.

    Layer-1 matmuls are K=64 and use only PE row strips 0-1, so two of them
    run CONCURRENTLY when placed at tile_position (0,0) and (64,0): the host
    interleaves the pair's two 500-row blocks across partition halves
    (block A features in partitions 0-63, block B in 64-127) and W1 is
    duplicated across both halves.  Each pass computes one h-half of block A
    alongside the other h-half of block B, so layer 1 costs ~1000 PE cycles
    per pair instead of 2000.  Everything else is v5: fp8 DoubleRow layer 2,
    (a) = 2 pair-level ACT relu+bias ops, (b) = ACT half0 / DVE half1 (DVE
    accumulates sum(max(ps2,-b2)); + b2*N lands in the host tail).
    """
    import concourse.mybir as mybir
    import concourse.tile as tile
    from concourse import bacc
    from contextlib import ExitStack

    dt = mybir.dt
    f32 = dt.float32
    f16 = dt.float16
    fp8 = dt.float8e4
    Relu = mybir.ActivationFunctionType.Relu
    Alu = mybir.AluOpType
    X = mybir.AxisListType.X

    R2 = R // 2  # columns of the interleaved x layout

    nc = bacc.Bacc(
        "TRN2",
        target_bir_lowering=False,
        debug=False,
        enable_asserts=False,
        num_devices=1,
    )

    d_xt = nc.dram_tensor("d_xt", [128, R2], f16, kind="ExternalInput").ap()
    d_w1 = nc.dram_tensor("d_w1", [128, H], f16, kind="ExternalInput").ap()
    d_w2 = nc.dram_tensor("d_w2", [128, 2, H], fp8, kind="ExternalInput").ap()
    d_b = nc.dram_tensor("d_b", [128, 4], f32, kind="ExternalInput").ap()
    d_s = nc.dram_tensor("d_s", [128, 2], f32, kind="ExternalOutput").ap()

    with tile.TileContext(nc) as tc, ExitStack() as ctx:
        cpool = ctx.enter_context(tc.tile_pool(name="cpool", bufs=1))
        xpool = ctx.enter_context(tc.tile_pool(name="xpool", bufs=3))
        hpool = ctx.enter_context(tc.tile_pool(name="hpool", bufs=2))
        spool = ctx.enter_context(tc.tile_pool(name="spool", bufs=1))
        ps1p = ctx.enter_context(tc.tile_pool(name="ps1p", bufs=1, space="PSUM"))
        ps2p = ctx.enter_context(tc.tile_pool(name="ps2p", bufs=1, space="PSUM"))

        # W1 duplicated across both partition halves: [128, 256] f16.
        w1_sb = cpool.tile([128, H], f16, name="w1_sb")
        nc.sync.dma_start(w1_sb[:], d_w1)
        w2p_sb = cpool.tile([128, 2, H], fp8, name="w2p_sb")
        nc.sync.dma_start(w2p_sb[:], d_w2)
        bv = cpool.tile([128, 4], f32, name="bv")
        nc.sync.dma_start(bv[:], d_b)
        nb2 = cpool.tile([128, 1], f32, name="nb2")
        nc.vector.tensor_scalar_mul(nb2[:], bv[:, 3:4], -1.0)

        acc = [cpool.tile([128, NPAIR], f32, name=f"acc{m}") for m in range(2)]

        for pair in [p for _ in range(iters) for p in range(NPAIR)]:
            # [128, 500]: partitions 0-63 = block A features, 64-127 = block B.
            xt = xpool.tile([128, BLK], f16, name="xt", tag="xt")
            nc.sync.dma_start(xt[:], d_xt[:, pair * BLK : (pair + 1) * BLK])

            ps1 = [
                ps1p.tile([128, 2, 512], f32, name=f"ps1_{m}", tag=f"ps1_{m}")
                for m in range(2)
            ]
            # Pass 1: A-h0 on strips 0-1 || B-h1 on strips 2-3.
            nc.tensor.matmul(
                ps1[0][:, 0, 0:BLK], w1_sb[0:64, 0:128], xt[0:64, 0:BLK],
                start=True, stop=True,
            )
            nc.tensor.matmul(
                ps1[1][:, 1, 0:BLK], w1_sb[64:128, 128:256], xt[64:128, 0:BLK],
                start=True, stop=True,
            )
            # Pass 2: A-h1 || B-h0.
            nc.tensor.matmul(
                ps1[1][:, 0, 0:BLK], w1_sb[0:64, 128:256], xt[0:64, 0:BLK],
                start=True, stop=True,
            )
            nc.tensor.matmul(
                ps1[0][:, 1, 0:BLK], w1_sb[64:128, 0:128], xt[64:128, 0:BLK],
                start=True, stop=True,
            )

            # (a): h1 = relu(ps1 + b1) -> packed fp8, one ACT op per half.
            h1 = hpool.tile([128, 2, 2, 512], fp8, name="h1", tag="h1")
            for m in range(2):
                nc.scalar.activation(
                    h1[:, m, :, 0:BLK],
                    ps1[m][:, :, 0:BLK],
                    Relu,
                    bias=bv[:, m : m + 1],
                )

            ps2 = [
                ps2p.tile([128, 2, 512], f32, name=f"ps2_{m}", tag=f"ps2_{m}")
                for m in range(2)
            ]
            for m in range(2):
                for j in range(2):
                    nc.tensor.matmul(
                        ps2[m][:, j, 0:BLK],
                        w2p_sb[:, :, m * 128 : (m + 1) * 128],
                        h1[:, :, j, 0:BLK],
                        start=True,
                        stop=True,
                        perf_mode=mybir.MatmulPerfMode.DoubleRow,
                    )

            scr0 = spool.tile([128, 2, 512], f16, name="scr0", tag="scr0")
            nc.scalar.activation(
                scr0[:, :, 0:BLK],
                ps2[0][:, :, 0:BLK],
                Relu,
                bias=bv[:, 2:3],
                accum_out=acc[0][:, pair : pair + 1],
            )
            scr1 = spool.tile([128, 2, 512], f16, name="scr1", tag="scr1")
            nc.vector.tensor_scalar(
                scr1[:, :, 0:BLK],
                ps2[1][:, :, 0:BLK],
                nb2[:],
                None,
                op0=Alu.max,
                op1=Alu.add,
                accum_out=acc[1][:, pair : pair + 1],
            )

        s_sb = cpool.tile([128, 2], f32, name="s_sb")
        for m in range(2):
            nc.vector.reduce_sum(s_sb[:, m : m + 1], acc[m][:], axis=X)
        nc.sync.dma_start(d_s, s_sb[:])

    nc.compile()
    return nc


def _build_base(mode: str, iters: int = 1, xbufs: int = 4, hbufs: int = 3):
    """The original staged baseline (f16 default): ones-row K=65 layer 1,
    f16 layer 2 in accumulation groups, DVE relu + 2 ACT relu+accum ops."""
    import concourse.mybir as mybir
    import concourse.tile as tile
    from concourse import bacc
    from contextlib import ExitStack

    dt = mybir.dt
    f32 = dt.float32
    split = mode == "f32r_split"
    mm_dt = {"f32r": dt.float32r, "f32r_split": dt.float32r, "f32": f32,
             "f16": dt.float16}[mode]

    nc = bacc.Bacc(
        "TRN2",
        target_bir_lowering=False,
        debug=False,
        enable_asserts=False,
        num_devices=1,
    )

    d_xt = nc.dram_tensor("d_xt", [IN + 1, R], mm_dt, kind="ExternalInput").ap()
    d_w1 = nc.dram_tensor("d_w1", [IN + 1, H], mm_dt, kind="ExternalInput").ap()
    d_w2 = nc.dram_tensor("d_w2", [H, H], mm_dt, kind="ExternalInput").ap()
    d_pb2 = nc.dram_tensor("d_pb2", [H], f32, kind="ExternalInput").ap()
    if split:
        d_w1l = nc.dram_tensor("d_w1l", [IN + 1, H], mm_dt, kind="ExternalInput").ap()
        d_w2l = nc.dram_tensor("d_w2l", [H, H], mm_dt, kind="ExternalInput").ap()
    d_s = nc.dram_tensor("d_s", [128, 2], f32, kind="ExternalOutput").ap()

    Relu = mybir.ActivationFunctionType.Relu
    X = mybir.AxisListType.X

    with tile.TileContext(nc) as tc, ExitStack() as ctx:
        cpool = ctx.enter_context(tc.tile_pool(name="cpool", bufs=1))
        xpool = ctx.enter_context(tc.tile_pool(name="xpool", bufs=xbufs))
        hpool = ctx.enter_context(tc.tile_pool(name="hpool", bufs=hbufs))
        spool = ctx.enter_context(tc.tile_pool(name="spool", bufs=2))
        ps1p = ctx.enter_context(tc.tile_pool(name="ps1p", bufs=2, space="PSUM"))
        ps2p = ctx.enter_context(tc.tile_pool(name="ps2p", bufs=2, space="PSUM"))

        w1_sb = cpool.tile([IN + 1, H], mm_dt, name="w1_sb")
        nc.sync.dma_start(w1_sb[:], d_w1)
        w2_sb = []
        for k in range(2):
            t = cpool.tile([128, H], mm_dt, name=f"w2_sb{k}")
            nc.sync.dma_start(t[:], d_w2[k * 128 : (k + 1) * 128, :])
            w2_sb.append(t)
        if split:
            w1l_sb = cpool.tile([IN + 1, H], mm_dt, name="w1l_sb")
            nc.sync.dma_start(w1l_sb[:], d_w1l)
            w2l_sb = []
            for k in range(2):
                t = cpool.tile([128, H], mm_dt, name=f"w2l_sb{k}")
                nc.sync.dma_start(t[:], d_w2l[k * 128 : (k + 1) * 128, :])
                w2l_sb.append(t)
        pb2_sb = cpool.tile([128, 2], f32, name="pb2_sb")
        nc.sync.dma_start(pb2_sb[:], d_pb2.rearrange("(m p) -> p m", p=128))

        acc = cpool.tile([128, 2, NBLK], f32, name="acc")

        for b in [b for _ in range(iters) for b in range(NBLK)]:
            xt = xpool.tile([IN + 1, BLK], mm_dt, name="xt", tag="xt")
            nc.sync.dma_start(xt[:], d_xt[:, b * BLK : (b + 1) * BLK])
            xr = xt[:]

            ps1 = ps1p.tile([128, 2, 512], f32, name="ps1", tag="ps1")
            for m in range(2):
                ms = slice(m * 128, (m + 1) * 128)
                nc.tensor.matmul(
                    ps1[:, m, 0:BLK], w1_sb[:, ms], xr,
                    start=True, stop=not split,
                )
                if split:
                    nc.tensor.matmul(
                        ps1[:, m, 0:BLK], w1l_sb[:, ms], xr,
                        start=False, stop=True,
                    )

            h1 = hpool.tile([128, 2, BLK], mm_dt, name="h1", tag="h1")
            nc.vector.tensor_scalar_max(h1[:], ps1[:, :, 0:BLK], 0.0)

            ps2 = ps2p.tile([128, 2, 512], f32, name="ps2", tag="ps2")
            for m in range(2):
                ms = slice(m * 128, (m + 1) * 128)
                mms = []
                for k in range(2):
                    mms.append((w2_sb[k][:, ms], h1[:, k, :]))
                    if split:
                        mms.append((w2l_sb[k][:, ms], h1[:, k, :]))
                for i, (lw, rr) in enumerate(mms):
                    nc.tensor.matmul(
                        ps2[:, m, 0:BLK], lw, rr,
                        start=(i == 0), stop=(i == len(mms) - 1),
                    )

            scr0 = spool.tile([128, BLK], f32, name="scr0", tag="scr0")
            nc.scalar.activation(
                scr0[:], ps2[:, 0, 0:BLK], Relu,
                bias=pb2_sb[:, 0:1],
                accum_out=acc[:, 0, b : b + 1],
            )
            scr1 = spool.tile([128, BLK], f32, name="scr1", tag="scr1")
            nc.scalar.activation(
                scr1[:], ps2[:, 1, 0:BLK], Relu,
                bias=pb2_sb[:, 1:2],
                accum_out=acc[:, 1, b : b + 1],
            )

        s_sb = cpool.tile([128, 2], f32, name="s_sb")
        nc.vector.reduce_sum(s_sb[:], acc[:], axis=X)
        nc.sync.dma_start(d_s, s_sb[:])

    nc.compile()
    return nc


def _hi_lo(w: np.ndarray):
    import ml_dtypes

    hi = np.asarray(w, dtype=ml_dtypes.bfloat16).astype(np.float32)
    lo = (w - hi).astype(np.float32)
    return hi, lo


def _diffuse_quant(W: np.ndarray, qdt) -> np.ndarray:
    """Error-diffusion quantization down the contraction axis: keeps
    per-column cumulative quantization error near zero so the (positive-mean)
    h1 stream doesn't see a systematic bias."""
    Wq = np.empty(W.shape, np.float32)
    carry = np.zeros(W.shape[1], np.float32)
    for k in range(W.shape[0]):
        t = W[k] + carry
        q = t.astype(qdt).astype(np.float32)
        carry = t - q
        Wq[k] = q
    return Wq


def _prep_in_maps(inputs: dict, mode: str):
    import ml_dtypes

    x = np.asarray(inputs["x"], dtype=np.float32)
    pw1 = np.asarray(inputs["pw1"], dtype=np.float32)
    pb1 = np.asarray(inputs["pb1"], dtype=np.float32)
    pw2 = np.asarray(inputs["pw2"], dtype=np.float32)
    pb2 = np.asarray(inputs["pb2"], dtype=np.float32)

    if mode in ("f16", "f32r", "f32r_split", "f32"):
        split = mode == "f32r_split"
        w1_aug = np.concatenate([pw1, pb1[None, :]], axis=0)  # [65, H]
        if split:
            w1h, w1l = _hi_lo(w1_aug)
            w2h, w2l = _hi_lo(pw2)
        else:
            w1h, w2h = w1_aug, pw2
        mm_np = np.float16 if mode == "f16" else np.float32
        w1h = w1h.astype(mm_np)
        w2h = w2h.astype(mm_np)
        in_maps = []
        for c in range(N_CORES):
            xt = np.empty((IN + 1, R), mm_np)
            xt[:IN] = x[c * R : (c + 1) * R].T.astype(mm_np)
            xt[IN] = 1.0
            m = {"d_xt": xt, "d_w1": w1h, "d_w2": w2h, "d_pb2": pb2}
            if split:
                m["d_w1l"] = w1l
                m["d_w2l"] = w2l
            in_maps.append(m)
        return in_maps

    fp8 = mode in ("v3fp8", "v4", "v5", "v6", "v7", "v8")
    if fp8:
        w2q = _diffuse_quant(pw2, ml_dtypes.float8_e4m3)
        w2 = np.ascontiguousarray(
            w2q.reshape(2, 128, H).transpose(1, 0, 2)
        ).astype(ml_dtypes.float8_e4m3)  # [k, pair, m]
    else:
        w2 = pw2.astype(np.float16)

    pw1h = pw1.astype(np.float16)
    common: dict
    if mode == "v4":
        b1m = np.zeros((128, 128), np.float16)
        b1m[64] = pb1[0:128].astype(np.float16)
        b1m[96] = pb1[128:256].astype(np.float16)
        b2m = np.stack([pb2[0:128], pb2[128:256]], axis=1).astype(np.float32)
        common = {"d_w1": pw1h, "d_w2": w2, "d_b1": b1m, "d_b2": b2m}
    elif mode == "v6":
        w1a = np.concatenate([pw1h, pb1[None, :].astype(np.float16)], axis=0)
        b2m = np.stack([pb2[0:128], pb2[128:256]], axis=1).astype(np.float32)
        common = {"d_w1": w1a, "d_w2": w2, "d_b2": b2m}
    elif mode in ("v7", "v8"):
        w1d = np.concatenate([pw1h, pw1h], axis=0)  # [128, 256]
        b = np.stack(
            [pb1[0:128], pb1[128:256], pb2[0:128], pb2[128:256]], axis=1
        ).astype(np.float32)
        common = {"d_w1": w1d, "d_w2": w2, "d_b": b}
    else:
        b = np.stack(
            [pb1[0:128], pb1[128:256], pb2[0:128], pb2[128:256]], axis=1
        ).astype(np.float32)  # [128, 4]
        common = {"d_w1": pw1h, "d_w2": w2, "d_b": b}

    in_maps = []
    for c in range(N_CORES):
        xc = x[c * R : (c + 1) * R].T.astype(np.float16)  # [64, R]
        if mode == "v6":
            xt = np.empty((IN + 1, R), np.float16)
            xt[:IN] = xc
            xt[IN] = 1.0
        elif mode in ("v7", "v8"):
            # interleave the pair's two 500-row blocks across partition
            # halves: [0:64] = even blocks, [64:128] = odd blocks.
            xr = xc.reshape(IN, NPAIR, 2, BLK)
            xt = np.concatenate(
                [
                    np.ascontiguousarray(xr[:, :, 0, :]).reshape(IN, R // 2),
                    np.ascontiguousarray(xr[:, :, 1, :]).reshape(IN, R // 2),
                ],
                axis=0,
            )  # [128, R//2]
        else:
            xt = np.ascontiguousarray(xc)
        in_maps.append({"d_xt": xt, **common})
    return in_maps


def _host_tail(S: np.ndarray, inputs: dict) -> np.ndarray:
    f = np.float64

    def g(name):
        return np.asarray(inputs[name], dtype=f)

    phi_sum = S @ g("pw3") + N * g("pb3")
    r = np.maximum(phi_sum @ g("rw1") + g("rb1"), 0.0)
    r = np.maximum(r @ g("rw2") + g("rb2"), 0.0)
    r = r @ g("rw3") + g("rb3")
    v = np.concatenate([r, g("x_static")])
    v = np.maximum(v @ g("w1") + g("b1"), 0.0)
    v = np.maximum(v @ g("w2") + g("b2"), 0.0)
    return (v @ g("w3") + g("b3")).astype(np.float32)


def _run(inputs: dict, trace: bool = False, mode: str | None = None):
    from concourse.bass_utils import run_bass_kernel_spmd

    mode = mode or MODE
    nc = _prog_cache.get(mode)
    if nc is None:
        nc = _build(mode)
        _prog_cache[mode] = nc

    if trace:
        try:
            import antenv.axon_hooks  # noqa: F401
        except ImportError:
            trace = False

    in_maps = _prep_in_maps(inputs, mode)
    res = run_bass_kernel_spmd(
        nc,
        in_maps,
        core_ids=list(range(N_CORES)),
        trace=trace,
    )

    S = np.zeros(H, np.float64)
    for rmap in res.results:
        s = rmap["d_s"].astype(np.float64)  # [128, 2]; channel = m*128 + p
        S += s.T.reshape(H)
    if mode in ("v4", "v5", "v6", "v7", "v8"):
        # the DVE path for h2-half1 accumulates sum(max(ps2, -b2)); the
        # + b2 * row-count shift is exact and lands here.
        S[128:256] += N * np.asarray(inputs["pb2"], np.float64)[128:256]
    out = _host_tail(S, inputs)
    return out, res


def kernel(**inputs) -> np.ndarray:
    out, _ = _run(inputs)
    return out
